# revision 50
# baseline (speedup 1.0000x reference)
"""Trainium2 Bass kernel for nn_AttentionLayer_85383949844589.

Gated attention layer: B=16, C=K=128, D=256.
  g0 = BN0(q @ W0.T)          per-C-channel stats over (B, D)
  g1 = BN1(kc @ W1.T)         per-K-channel stats over (B, D)
  aw[b,c,k,d]   = sigmoid(g1)[b,k,d] * sigmoid(g0)[b,c,d]
  attn[b,c,k,d] = kc[b,k,d] * aw * cmask[b,c] * kmask[b,k]
  out[b,c,d]    = tanh(sum_k attn / klen[b])
  awm[b,c,k]    = mean_d aw

Sharding: the C (query-channel) axis is split across the 8 NeuronCores
(16 channels each).  BN0 stats are per-C-channel, so they are fully local
to a core; the g1/BN1 pipeline is replicated on every core (it is tiny).
No cross-core communication is needed at all.

Per core the dominant cost is writing its (B, C/8, K, D) = 32 MiB slice of
attn, i.e. the kernel is HBM-write-bound (~95 us at ~358 GB/s/core).

The big product is computed with D on the partition axis:
  big_t[d, c, k] = QgT[d, c] * A_t[d, k]
where QgT = sigmoid(g0)*cmask transposed and A_t = sigmoid(g1)T * (kc*kmask)T.
In that layout BOTH operands of the (C/8 x K) outer product are plain
free-dimension broadcast views (stride-0 free dims), so the DVE computes the
whole 4D block with two tensor_tensor ops per batch — no partition broadcast
is needed anywhere.  As a bonus the (d-partition, (c,k)-free) store has
8 KB-contiguous DRAM runs (vs 1 KB for the natural layout); the host
re-transposes the (B, 2, 128, C/8, K) device output once at the end.

Phase 1 (Y = x@W.T + per-channel sum/sumsq) runs on PE/scalar/vector with a
dep-free bf16 warm-up burst to lift the PE HAM throttle; phase 2 alternates
the two HWDGE queues so the 2 MB stores never head-of-line-block the
pipeline's small DMAs.  BN finalize runs split across scalar (BN1) and
vector (BN0) so the two serial chains overlap.
"""

import sys

sys.path.insert(0, "/opt/trn_rl_repo")

import numpy as np

B, C, K, D = 16, 128, 128, 256
NCORES = 8
CL = C // NCORES  # 16 query channels per core
EPS = 1e-5

_CACHE: dict = {}


def _build_nc():
    import concourse.tile as tile
    from concourse import bacc, mybir

    fp32 = mybir.dt.float32
    AF = mybir.ActivationFunctionType
    OP = mybir.AluOpType
    AX = mybir.AxisListType

    nc = bacc.Bacc(trn_type="TRN2", debug=False, num_devices=NCORES)

    # ---- DRAM I/O ----
    # qt[p, b, h, c]  = q[b, c_slice[c], h*128+p]
    qt_d = nc.dram_tensor("qt", [128, B, 2, CL], fp32, kind="ExternalInput")
    # kct[p, b, h, k] = kc[b, k, h*128+p]
    kct_d = nc.dram_tensor("kct", [128, B, 2, K], fp32, kind="ExternalInput")
    # kcm[p, b, h, k] = kc[b, k, h*128+p] * kmask[b, k]
    kcm_d = nc.dram_tensor("kcm", [128, B, 2, K], fp32, kind="ExternalInput")
    # wXt[p, h, o]    = WX[o, h*128+p]
    w0t_d = nc.dram_tensor("w0t", [128, 2, D], fp32, kind="ExternalInput")
    w1t_d = nc.dram_tensor("w1t", [128, 2, D], fp32, kind="ExternalInput")
    # all small constants packed into one tensor (single DMA):
    # cols [0]=g1, [1]=b1, [2]=g0(rows 0:CL), [3]=b0(rows 0:CL),
    # [4:4+B]=ilen(128,B), [20:20+B]=cmt(rows 0:CL), [36:164]=identity
    cpk_d = nc.dram_tensor("cpk", [128, 164], fp32, kind="ExternalInput")

    # transposed layouts (d on partitions); host reassembles
    ores_d = nc.dram_tensor("o_res", [B, 2, 128, CL], fp32, kind="ExternalOutput")
    attn_d = nc.dram_tensor("o_attn", [B, 2, 128, CL, K], fp32, kind="ExternalOutput")
    awm_d = nc.dram_tensor("o_awm", [B, CL, K], fp32, kind="ExternalOutput")

    BD = float(B * D)

    with tile.TileContext(nc) as tc:
        with (
            tc.tile_pool(name="const", bufs=1) as cp,
            tc.tile_pool(name="persist", bufs=1) as pp,
            tc.tile_pool(name="stats", bufs=1) as sp,
            tc.tile_pool(name="work", bufs=2) as wp,
            tc.tile_pool(name="bigout", bufs=4) as bp,
        ):
            # ---- load inputs; kct/kcm/w first so phase 1 starts ASAP ----
            kct = pp.tile([128, B * 2 * K], fp32)
            kcm = pp.tile([128, B * 2 * K], fp32)
            CH = B // 4

            def load_kct(i):
                eng = nc.sync if i % 2 == 0 else nc.scalar
                sl_d = slice(i * CH, (i + 1) * CH)
                sl_s = slice(i * CH * 2 * K, (i + 1) * CH * 2 * K)
                eng.dma_start(
                    kct[:, sl_s],
                    kct_d.ap()[:, sl_d].rearrange("p b h k -> p (b h k)"),
                )

            def load_kcm(i):
                eng = nc.sync if i % 2 == 0 else nc.scalar
                sl_d = slice(i * CH, (i + 1) * CH)
                sl_s = slice(i * CH * 2 * K, (i + 1) * CH * 2 * K)
                eng.dma_start(
                    kcm[:, sl_s],
                    kcm_d.ap()[:, sl_d].rearrange("p b h k -> p (b h k)"),
                )

            load_kct(0)
            load_kct(1)
            w1t = cp.tile([128, 2 * D], fp32)
            nc.sync.dma_start(w1t[:], w1t_d.ap().rearrange("p h o -> p (h o)"))
            w0t = cp.tile([128, 2 * D], fp32)
            nc.scalar.dma_start(w0t[:], w0t_d.ap().rearrange("p h o -> p (h o)"))
            load_kct(2)
            load_kct(3)
            qt = pp.tile([128, B * 2 * CL], fp32)
            nc.scalar.dma_start(qt[:], qt_d.ap().rearrange("p b h c -> p (b h c)"))
            cpk = cp.tile([128, 164], fp32)
            nc.sync.dma_start(cpk[:], cpk_d.ap()[:])
            for i in range(4):
                load_kcm(i)
            g1c = cpk[:, 0:1]
            b1c = cpk[:, 1:2]
            g0c = cpk[0:CL, 2:3]
            b0c = cpk[0:CL, 3:4]
            ilen = cpk[:, 4 : 4 + B]
            cmt = cpk[0:CL, 20 : 20 + B]
            iden = cpk[:, 36:164]

            y1sb = pp.tile([K, B * D], fp32)
            y0sb = pp.tile([CL, B * D], fp32)

            s1cols = sp.tile([K, B], fp32)
            q1cols = sp.tile([K, B], fp32)
            s0cols = sp.tile([CL, B], fp32)
            q0cols = sp.tile([CL, B], fp32)
            sq1s = sp.tile([K, 2 * D], fp32)
            sq0s = sp.tile([CL, 2 * D], fp32)
            epst = sp.tile([128, 1], fp32)
            nc.vector.memset(epst[:], EPS)

            # ---- PE warm-up burst ----
            # HAM starts the PE throttled (1.2 GHz) and only un-throttles
            # after a sustained-busy window.  Run dep-free bf16 matmuls while
            # the input DMAs land so phase 1 runs at 2.4 GHz.
            bf16 = mybir.dt.bfloat16
            wu_a = sp.tile([128, 128], bf16)
            nc.vector.memset(wu_a[:], 1.0)
            wu_b = sp.tile([128, 512], bf16)
            nc.vector.memset(wu_b[:], 1.0)
            with tc.tile_pool(name="pswu", bufs=1, space="PSUM") as pswu:
                wu_ps = pswu.tile([128, 512], fp32)
                NWU = 12
                for i in range(NWU):
                    nc.tensor.matmul(
                        wu_ps[:], wu_a[:], wu_b[:],
                        start=(i == 0), stop=(i == NWU - 1),
                    )
                wu_out = sp.tile([1, 1], fp32)
                nc.scalar.copy(wu_out[:], wu_ps[0:1, 0:1])

            # ---- phase 1: Y0/Y1 matmuls + per-channel sum / sumsq ----
            with tc.tile_pool(name="ps1", bufs=3, space="PSUM") as ps1:
                for b in range(B):
                    y1ps = ps1.tile([K, D], fp32, tag="y1ps")
                    for h in range(2):
                        nc.tensor.matmul(
                            y1ps[:],
                            kct[:, b * 256 + h * 128 : b * 256 + h * 128 + 128],
                            w1t[:, h * D : (h + 1) * D],
                            start=(h == 0),
                            stop=(h == 1),
                        )
                    nc.scalar.copy(y1sb[:, b * D : (b + 1) * D], y1ps[:])

                    y0ps = ps1.tile([CL, D], fp32, tag="y0ps")
                    for h in range(2):
                        nc.tensor.matmul(
                            y0ps[:],
                            qt[:, b * 2 * CL + h * CL : b * 2 * CL + (h + 1) * CL],
                            w0t[:, h * D : (h + 1) * D],
                            start=(h == 0),
                            stop=(h == 1),
                        )
                    nc.scalar.copy(y0sb[:, b * D : (b + 1) * D], y0ps[:])

                    # per-2-batch channel stats on the vector engine, from
                    # the SBUF copies (vector must not touch live PSUM here)
                    if b % 2 == 1:
                        i4 = b // 2
                        c0, c1 = (b - 1) * D, (b + 1) * D
                        nc.vector.tensor_reduce(
                            s1cols[:, i4 : i4 + 1], y1sb[:, c0:c1], AX.X, OP.add
                        )
                        nc.vector.tensor_mul(sq1s[:], y1sb[:, c0:c1], y1sb[:, c0:c1])
                        nc.vector.tensor_reduce(
                            q1cols[:, i4 : i4 + 1], sq1s[:], AX.X, OP.add
                        )
                        nc.vector.tensor_reduce(
                            s0cols[:, i4 : i4 + 1], y0sb[:, c0:c1], AX.X, OP.add
                        )
                        nc.vector.tensor_mul(sq0s[:], y0sb[:, c0:c1], y0sb[:, c0:c1])
                        nc.vector.tensor_reduce(
                            q0cols[:, i4 : i4 + 1], sq0s[:], AX.X, OP.add
                        )

            # ---- phase boundary: finalize BN scale/shift ----
            # s = gamma / sqrt(var+eps);  t = beta - mean * s
            # Mostly on the scalar engine: DVE ops pay a pipeline DRAIN each,
            # which dominates this serial chain of tiny (P,1) ops.
            def bn_finalize(P, scols, qcols, gc, bc):
                ssum = sp.tile([P, 1], fp32, name=f"ssum{P}")
                nc.vector.tensor_reduce(ssum[:], scols[:, 0:8], AX.X, OP.add)
                qsum = sp.tile([P, 1], fp32, name=f"qsum{P}")
                nc.vector.tensor_reduce(qsum[:], qcols[:, 0:8], AX.X, OP.add)
                mean = sp.tile([P, 1], fp32, name=f"mean{P}")
                nc.scalar.mul(mean[:], ssum[:], 1.0 / BD)
                # ex2e = E[x^2] + eps
                ex2e = sp.tile([P, 1], fp32, name=f"ex2e{P}")
                nc.scalar.activation(
                    ex2e[:], qsum[:], AF.Identity, bias=epst[:P], scale=1.0 / BD
                )
                msq = sp.tile([P, 1], fp32, name=f"msq{P}")
                nc.scalar.square(msq[:], mean[:])
                # varp = ex2e - mean^2
                varp = sp.tile([P, 1], fp32, name=f"varp{P}")
                nc.scalar.activation(
                    varp[:], msq[:], AF.Identity, bias=ex2e[:], scale=-1.0
                )
                std = sp.tile([P, 1], fp32, name=f"std{P}")
                nc.scalar.sqrt(std[:], varp[:])
                # one Newton step to clean up the scalar-engine sqrt:
                # std' = 0.5*(std + varp/std)
                rstd = sp.tile([P, 1], fp32, name=f"rstd{P}")
                nc.vector.reciprocal(rstd[:], std[:])
                qh = sp.tile([P, 1], fp32, name=f"qh{P}")
                nc.scalar.mul(qh[:], varp[:], rstd[:])  # varp/std
                stdh = sp.tile([P, 1], fp32, name=f"stdh{P}")
                nc.scalar.mul(stdh[:], std[:], 0.5)
                std2 = sp.tile([P, 1], fp32, name=f"std2{P}")
                nc.scalar.activation(
                    std2[:], qh[:], AF.Identity, bias=stdh[:], scale=0.5
                )
                inv = sp.tile([P, 1], fp32, name=f"inv{P}")
                nc.vector.reciprocal(inv[:], std2[:])
                s_ = sp.tile([P, 1], fp32, name=f"s_{P}")
                nc.scalar.mul(s_[:], inv[:], gc[:])
                ms = sp.tile([P, 1], fp32, name=f"ms{P}")
                nc.scalar.mul(ms[:], mean[:], s_[:])
                t_ = sp.tile([P, 1], fp32, name=f"t_{P}")
                nc.scalar.activation(
                    t_[:], ms[:], AF.Identity, bias=bc[:], scale=-1.0
                )
                return s_, t_

            def bn_finalize_v(P, scols, qcols, gc, bc):
                # vector-engine variant so BN0 finalizes concurrently with
                # BN1 on the scalar engine
                ssum = sp.tile([P, 1], fp32, name=f"vssum{P}")
                nc.vector.tensor_reduce(ssum[:], scols[:, 0:8], AX.X, OP.add)
                qsum = sp.tile([P, 1], fp32, name=f"vqsum{P}")
                nc.vector.tensor_reduce(qsum[:], qcols[:, 0:8], AX.X, OP.add)
                mean = sp.tile([P, 1], fp32, name=f"vmean{P}")
                nc.vector.tensor_scalar_mul(mean[:], ssum[:], 1.0 / BD)
                ex2e = sp.tile([P, 1], fp32, name=f"vex2e{P}")
                nc.vector.tensor_scalar(
                    ex2e[:], qsum[:], 1.0 / BD, EPS, OP.mult, OP.add
                )
                msq = sp.tile([P, 1], fp32, name=f"vmsq{P}")
                nc.vector.tensor_mul(msq[:], mean[:], mean[:])
                varp = sp.tile([P, 1], fp32, name=f"vvarp{P}")
                nc.vector.tensor_sub(varp[:], ex2e[:], msq[:])
                std = sp.tile([P, 1], fp32, name=f"vstd{P}")
                nc.scalar.sqrt(std[:], varp[:])
                rstd = sp.tile([P, 1], fp32, name=f"vrstd{P}")
                nc.vector.reciprocal(rstd[:], std[:])
                q_ = sp.tile([P, 1], fp32, name=f"vq_{P}")
                nc.vector.tensor_mul(q_[:], varp[:], rstd[:])
                nc.vector.tensor_add(std[:], std[:], q_[:])
                nc.vector.tensor_scalar_mul(std[:], std[:], 0.5)
                inv = sp.tile([P, 1], fp32, name=f"vinv{P}")
                nc.vector.reciprocal(inv[:], std[:])
                s_ = sp.tile([P, 1], fp32, name=f"vs_{P}")
                nc.vector.tensor_mul(s_[:], inv[:], gc[:])
                ms = sp.tile([P, 1], fp32, name=f"vms{P}")
                nc.vector.tensor_mul(ms[:], mean[:], s_[:])
                t_ = sp.tile([P, 1], fp32, name=f"vt_{P}")
                nc.vector.tensor_sub(t_[:], bc[:], ms[:])
                return s_, t_

            s1, t1 = bn_finalize(K, s1cols, q1cols, g1c, b1c)
            s0, t0 = bn_finalize_v(CL, s0cols, q0cols, g0c, b0c)

            # Bake cmask into a per-(c,b) scale/bias so Qg = sigmoid-masked
            # comes straight off the scalar engine:
            #   masked: sigmoid(s0*y + t0);  unmasked: sigmoid(0*y - 1e30) = 0
            s0b = sp.tile([CL, B], fp32)
            nc.vector.tensor_scalar(s0b[:], cmt[:], s0[:], None, OP.mult)
            t0b = sp.tile([CL, B], fp32)
            # t0b = t0*cm + (cm-1)*1e30
            nc.vector.tensor_scalar(t0b[:], cmt[:], 1.0, 1e30, OP.subtract, OP.mult)
            tb2 = sp.tile([CL, B], fp32)
            nc.vector.tensor_scalar(tb2[:], cmt[:], t0[:], None, OP.mult)
            nc.vector.tensor_add(t0b[:], t0b[:], tb2[:])

            # ---- phase 2 ----
            # PSUM transpose-staging layout (single bank):
            #   [  0:128) sig1T h0   [128:256) sig1T h1
            #   [256:272) sig0T h0   [272:288) sig0T h1
            #   [288:304) QgT  h0    [304:320) QgT  h1
            S1T, S0T, QGT = 0, 256, 288
            with (
                tc.tile_pool(name="pst", bufs=2, space="PSUM") as pst,  # transposes
                tc.tile_pool(name="psr", bufs=2, space="PSUM") as psr,  # awm
            ):
                for b in range(B):
                    # alternate HWDGE queues so the big output DMA never
                    # head-of-line-blocks the small pipeline DMAs
                    dq = nc.sync if (b % 2 == 0) else nc.scalar
                    oq = nc.scalar if (b % 2 == 0) else nc.sync

                    yb = y1sb[:, b * D : (b + 1) * D]
                    sig1 = wp.tile([K, D], fp32, tag="sig1")
                    nc.scalar.activation(
                        sig1[:], yb, AF.Sigmoid, bias=t1[:], scale=s1[:]
                    )

                    sig0 = wp.tile([CL, D], fp32, tag="sig0")
                    nc.scalar.activation(
                        sig0[:],
                        y0sb[:, b * D : (b + 1) * D],
                        AF.Sigmoid,
                        bias=t0[:],
                        scale=s0[:],
                    )
                    qg = wp.tile([CL, D], fp32, tag="qg")
                    nc.scalar.activation(
                        qg[:],
                        y0sb[:, b * D : (b + 1) * D],
                        AF.Sigmoid,
                        bias=t0b[:, b : b + 1],
                        scale=s0b[:, b : b + 1],
                    )

                    # transpose into d-on-partitions layout
                    tps = pst.tile([128, 320], fp32, tag="tps")
                    for h in range(2):
                        nc.tensor.transpose(
                            tps[:, S1T + h * K : S1T + (h + 1) * K],
                            sig1[:, h * 128 : (h + 1) * 128],
                            iden[:, 0:128],
                        )
                        nc.tensor.transpose(
                            tps[:, S0T + h * CL : S0T + (h + 1) * CL],
                            sig0[:, h * 128 : (h + 1) * 128],
                            iden[0:CL, 0:CL],
                        )
                        nc.tensor.transpose(
                            tps[:, QGT + h * CL : QGT + (h + 1) * CL],
                            qg[:, h * 128 : (h + 1) * 128],
                            iden[0:CL, 0:CL],
                        )
                    st = wp.tile([128, 320], fp32, tag="st")
                    nc.scalar.copy(st[:], tps[:])


                    # A_t[d, k] = sig1T[d, k] * (kc*kmask)T[d, k];
                    # accum_out gives sum_k A_t = the attention-vector sum
                    at2 = wp.tile([128, 2 * K], fp32, tag="at2")
                    sA = wp.tile([128, 2], fp32, tag="sA")
                    for h in range(2):
                        nc.vector.scalar_tensor_tensor(
                            at2[:, h * K : (h + 1) * K],
                            st[:, S1T + h * K : S1T + (h + 1) * K],
                            1.0,
                            kcm[:, b * 2 * K + h * K : b * 2 * K + (h + 1) * K],
                            op0=OP.bypass,
                            op1=OP.mult,
                            accum_out=sA[:, h : h + 1],
                        )

                    # awm[c,k] = (1/D) * sum_d sig0T[d,c] * sig1T[d,k]
                    psr_t = psr.tile([CL, K], fp32, tag="psr")
                    for h in range(2):
                        nc.tensor.matmul(
                            psr_t[:],
                            st[:, S0T + h * CL : S0T + (h + 1) * CL],
                            st[:, S1T + h * K : S1T + (h + 1) * K],
                            start=(h == 0),
                            stop=(h == 1),
                        )
                    awm_sb = wp.tile([CL, K], fp32, tag="awm_sb")
                    nc.scalar.mul(awm_sb[:], psr_t[:], 1.0 / D)
                    oq.dma_start(awm_d.ap()[b], awm_sb[:])

                    # attention_vector (transposed): sum_k A_t along free,
                    # then av_t[d,c] = QgT[d,c] * sumA[d], tanh(av/klen)
                    av_t = wp.tile([128, 2 * CL], fp32, tag="av_t")
                    for h in range(2):
                        nc.scalar.mul(
                            av_t[:, h * CL : (h + 1) * CL],
                            st[:, QGT + h * CL : QGT + (h + 1) * CL],
                            sA[:, h : h + 1],
                        )
                    ores_t = wp.tile([128, 2 * CL], fp32, tag="ores_t")
                    nc.scalar.activation(
                        ores_t[:], av_t[:], AF.Tanh, bias=0.0, scale=ilen[:, b : b + 1]
                    )
                    oq.dma_start(
                        ores_d.ap()[b].rearrange("h p c -> p h c"),
                        ores_t[:].rearrange("p (h c) -> p h c", c=CL),
                    )

                    # big product, d on partitions:
                    #   big_t[d, c, k] = QgT[d, c] * A_t[d, k]
                    big = bp.tile([128, 2 * CL * K], fp32, tag="big")
                    nc.vector.tensor_tensor(
                        big[:].rearrange("p (h c k) -> p h c k", c=CL, k=K),
                        st[:, QGT : QGT + 2 * CL]
                        .rearrange("p (h c) -> p h c", c=CL)
                        .unsqueeze(3)
                        .to_broadcast([128, 2, CL, K]),
                        at2[:]
                        .rearrange("p (h k) -> p h k", k=K)
                        .unsqueeze(2)
                        .to_broadcast([128, 2, CL, K]),
                        OP.mult,
                    )
                    dq.dma_start(
                        attn_d.ap()[b].rearrange("h p c k -> p h c k"),
                        big[:].rearrange("p (h c k) -> p h c k", c=CL, k=K),
                    )

    nc.compile()
    return nc


def _get_nc():
    if "nc" not in _CACHE:
        _CACHE["nc"] = _build_nc()
    return _CACHE["nc"]


def _make_in_maps(inputs):
    q = np.ascontiguousarray(inputs["query_candidates_repr"], dtype=np.float32)
    kc = np.ascontiguousarray(inputs["key_candidates"], dtype=np.float32)
    W0 = np.asarray(inputs["W0"], dtype=np.float32)
    W1 = np.asarray(inputs["W1"], dtype=np.float32)
    g0 = np.asarray(inputs["bn0_gamma"], dtype=np.float32)
    b0 = np.asarray(inputs["bn0_beta"], dtype=np.float32)
    g1 = np.asarray(inputs["bn1_gamma"], dtype=np.float32)
    b1 = np.asarray(inputs["bn1_beta"], dtype=np.float32)
    cm = np.asarray(inputs["query_candidate_mask"]).astype(np.float32)
    km = np.asarray(inputs["key_candidate_mask"]).astype(np.float32)
    kl = np.asarray(inputs["key_candidate_len"]).astype(np.float32)

    kct = np.ascontiguousarray(
        kc.reshape(B, K, 2, 128).transpose(3, 0, 2, 1)
    )  # (128, B, 2, K)
    kcm = np.ascontiguousarray(
        (kc * km[:, :, None]).reshape(B, K, 2, 128).transpose(3, 0, 2, 1)
    )  # (128, B, 2, K), kmask folded in
    w0t = np.ascontiguousarray(W0.reshape(D, 2, 128).transpose(2, 1, 0))
    w1t = np.ascontiguousarray(W1.reshape(D, 2, 128).transpose(2, 1, 0))

    shared = dict(kct=kct, kcm=kcm, w0t=w0t, w1t=w1t)
    in_maps = []
    for r in range(NCORES):
        sl = slice(r * CL, (r + 1) * CL)
        qt = np.ascontiguousarray(
            q[:, sl, :].reshape(B, CL, 2, 128).transpose(3, 0, 2, 1)
        )
        cpk = np.zeros((128, 164), np.float32)
        cpk[:, 0] = g1
        cpk[:, 1] = b1
        cpk[:CL, 2] = g0[sl]
        cpk[:CL, 3] = b0[sl]
        cpk[:, 4 : 4 + B] = np.tile(1.0 / kl, (128, 1))
        cpk[:CL, 20 : 20 + B] = cm[:, sl].T
        cpk[:, 36:164] = np.eye(128, dtype=np.float32)
        m = dict(shared, qt=qt, cpk=cpk)
        in_maps.append(m)
    return in_maps


def run(inputs, trace=False):
    from concourse import bass_utils

    nc = _get_nc()
    in_maps = _make_in_maps(inputs)
    res = bass_utils.run_bass_kernel_spmd(
        nc, in_maps, core_ids=list(range(NCORES)), trace=trace
    )
    # device outputs are d-on-partitions (B, 2, 128, CL[, K]); restore layout
    ores_t = np.stack([res.results[r]["o_res"] for r in range(NCORES)], axis=3)
    # (B, 2, 128, NCORES, CL) -> (B, C, D)
    out_res = np.ascontiguousarray(
        ores_t.transpose(0, 3, 4, 1, 2).reshape(B, C, D)
    )
    attn_t = np.stack([res.results[r]["o_attn"] for r in range(NCORES)], axis=3)
    # (B, 2, 128, NCORES, CL, K) -> (B, C, K, D)
    attn = np.ascontiguousarray(
        attn_t.transpose(0, 3, 4, 5, 1, 2).reshape(B, C, K, D)
    )
    awm = np.concatenate([res.results[r]["o_awm"] for r in range(NCORES)], axis=1)
    return (out_res, attn, awm), res


def kernel(**inputs):
    (out_res, attn, awm), _ = run(inputs, trace=False)
    return out_res, attn, awm


# revision 51
# speedup vs baseline: 1.0096x; 1.0096x over previous
"""Trainium2 Bass kernel for nn_AttentionLayer_85383949844589.

Gated attention layer: B=16, C=K=128, D=256.
  g0 = BN0(q @ W0.T)          per-C-channel stats over (B, D)
  g1 = BN1(kc @ W1.T)         per-K-channel stats over (B, D)
  aw[b,c,k,d]   = sigmoid(g1)[b,k,d] * sigmoid(g0)[b,c,d]
  attn[b,c,k,d] = kc[b,k,d] * aw * cmask[b,c] * kmask[b,k]
  out[b,c,d]    = tanh(sum_k attn / klen[b])
  awm[b,c,k]    = mean_d aw

Sharding: the C (query-channel) axis is split across the 8 NeuronCores
(16 channels each).  BN0 stats are per-C-channel, so they are fully local
to a core; the g1/BN1 pipeline is replicated on every core (it is tiny).
No cross-core communication is needed at all.

Per core the dominant cost is writing its (B, C/8, K, D) = 32 MiB slice of
attn, i.e. the kernel is HBM-write-bound (~95 us at ~358 GB/s/core).

The big product is computed with D on the partition axis:
  big_t[d, c, k] = QgT[d, c] * A_t[d, k]
where QgT = sigmoid(g0)*cmask transposed and A_t = sigmoid(g1)T * (kc*kmask)T.
In that layout BOTH operands of the (C/8 x K) outer product are plain
free-dimension broadcast views (stride-0 free dims), so the DVE computes the
whole 4D block with two tensor_tensor ops per batch — no partition broadcast
is needed anywhere.  As a bonus the (d-partition, (c,k)-free) store has
8 KB-contiguous DRAM runs (vs 1 KB for the natural layout); the host
re-transposes the (B, 2, 128, C/8, K) device output once at the end.

Phase 1 (Y = x@W.T + per-channel sum/sumsq) runs on PE/scalar/vector with a
dep-free bf16 warm-up burst to lift the PE HAM throttle; phase 2 alternates
the two HWDGE queues so the 2 MB stores never head-of-line-block the
pipeline's small DMAs.  BN finalize runs split across scalar (BN1) and
vector (BN0) so the two serial chains overlap.
"""

import sys

sys.path.insert(0, "/opt/trn_rl_repo")

import numpy as np

B, C, K, D = 16, 128, 128, 256
NCORES = 8
CL = C // NCORES  # 16 query channels per core
EPS = 1e-5

_CACHE: dict = {}


def _build_nc():
    import concourse.tile as tile
    from concourse import bacc, mybir

    fp32 = mybir.dt.float32
    AF = mybir.ActivationFunctionType
    OP = mybir.AluOpType
    AX = mybir.AxisListType

    nc = bacc.Bacc(trn_type="TRN2", debug=False, num_devices=NCORES)

    # ---- DRAM I/O ----
    # qt[p, b, h, c]  = q[b, c_slice[c], h*128+p]
    qt_d = nc.dram_tensor("qt", [128, B, 2, CL], fp32, kind="ExternalInput")
    # kct[p, b, h, k] = kc[b, k, h*128+p]
    kct_d = nc.dram_tensor("kct", [128, B, 2, K], fp32, kind="ExternalInput")
    # kcm[p, b, h, k] = kc[b, k, h*128+p] * kmask[b, k]
    kcm_d = nc.dram_tensor("kcm", [128, B, 2, K], fp32, kind="ExternalInput")
    # wXt[p, h, o]    = WX[o, h*128+p]
    w0t_d = nc.dram_tensor("w0t", [128, 2, D], fp32, kind="ExternalInput")
    w1t_d = nc.dram_tensor("w1t", [128, 2, D], fp32, kind="ExternalInput")
    # all small constants packed into one tensor (single DMA):
    # cols [0]=g1, [1]=b1, [2]=g0(rows 0:CL), [3]=b0(rows 0:CL),
    # [4:4+B]=ilen(128,B), [20:20+B]=cmt(rows 0:CL), [36:164]=identity
    cpk_d = nc.dram_tensor("cpk", [128, 164], fp32, kind="ExternalInput")

    # transposed layouts (d on partitions); host reassembles
    ores_d = nc.dram_tensor("o_res", [B, 2, 128, CL], fp32, kind="ExternalOutput")
    attn_d = nc.dram_tensor("o_attn", [B, 2, 128, CL, K], fp32, kind="ExternalOutput")
    awm_d = nc.dram_tensor("o_awm", [B, CL, K], fp32, kind="ExternalOutput")

    BD = float(B * D)

    with tile.TileContext(nc) as tc:
        with (
            tc.tile_pool(name="const", bufs=1) as cp,
            tc.tile_pool(name="persist", bufs=1) as pp,
            tc.tile_pool(name="stats", bufs=1) as sp,
            tc.tile_pool(name="work", bufs=2) as wp,
            tc.tile_pool(name="bigout", bufs=4) as bp,
        ):
            # ---- load inputs; kct/kcm/w first so phase 1 starts ASAP ----
            kct = pp.tile([128, B * 2 * K], fp32)
            kcm = pp.tile([128, B * 2 * K], fp32)
            CH = B // 4

            def load_kct(i):
                eng = nc.sync if i % 2 == 0 else nc.scalar
                sl_d = slice(i * CH, (i + 1) * CH)
                sl_s = slice(i * CH * 2 * K, (i + 1) * CH * 2 * K)
                eng.dma_start(
                    kct[:, sl_s],
                    kct_d.ap()[:, sl_d].rearrange("p b h k -> p (b h k)"),
                )

            def load_kcm(i):
                eng = nc.sync if i % 2 == 0 else nc.scalar
                sl_d = slice(i * CH, (i + 1) * CH)
                sl_s = slice(i * CH * 2 * K, (i + 1) * CH * 2 * K)
                eng.dma_start(
                    kcm[:, sl_s],
                    kcm_d.ap()[:, sl_d].rearrange("p b h k -> p (b h k)"),
                )

            load_kct(0)
            load_kct(1)
            w1t = cp.tile([128, 2 * D], fp32)
            nc.sync.dma_start(w1t[:], w1t_d.ap().rearrange("p h o -> p (h o)"))
            w0t = cp.tile([128, 2 * D], fp32)
            nc.scalar.dma_start(w0t[:], w0t_d.ap().rearrange("p h o -> p (h o)"))
            load_kct(2)
            load_kct(3)
            qt = pp.tile([128, B * 2 * CL], fp32)
            nc.scalar.dma_start(qt[:], qt_d.ap().rearrange("p b h c -> p (b h c)"))
            cpk = cp.tile([128, 164], fp32)
            nc.sync.dma_start(cpk[:], cpk_d.ap()[:])
            for i in range(4):
                load_kcm(i)
            g1c = cpk[:, 0:1]
            b1c = cpk[:, 1:2]
            g0c = cpk[0:CL, 2:3]
            b0c = cpk[0:CL, 3:4]
            ilen = cpk[:, 4 : 4 + B]
            cmt = cpk[0:CL, 20 : 20 + B]
            iden = cpk[:, 36:164]

            y1sb = pp.tile([K, B * D], fp32)
            y0sb = pp.tile([CL, B * D], fp32)

            s1cols = sp.tile([K, B], fp32)
            q1cols = sp.tile([K, B], fp32)
            s0cols = sp.tile([CL, B], fp32)
            q0cols = sp.tile([CL, B], fp32)
            sq1s = sp.tile([K, 2 * D], fp32)
            sq0s = sp.tile([CL, 2 * D], fp32)
            epst = sp.tile([128, 1], fp32)
            nc.vector.memset(epst[:], EPS)
            ores_all = pp.tile([128, B * 2 * CL], fp32)
            awm_all = pp.tile([CL, B * K], fp32)

            # ---- PE warm-up burst ----
            # HAM starts the PE throttled (1.2 GHz) and only un-throttles
            # after a sustained-busy window.  Run dep-free bf16 matmuls while
            # the input DMAs land so phase 1 runs at 2.4 GHz.
            bf16 = mybir.dt.bfloat16
            wu_a = sp.tile([128, 128], bf16)
            nc.vector.memset(wu_a[:], 1.0)
            wu_b = sp.tile([128, 512], bf16)
            nc.vector.memset(wu_b[:], 1.0)
            with tc.tile_pool(name="pswu", bufs=1, space="PSUM") as pswu:
                wu_ps = pswu.tile([128, 512], fp32)
                NWU = 8
                for i in range(NWU):
                    nc.tensor.matmul(
                        wu_ps[:], wu_a[:], wu_b[:],
                        start=(i == 0), stop=(i == NWU - 1),
                    )
                wu_out = sp.tile([1, 1], fp32)
                nc.scalar.copy(wu_out[:], wu_ps[0:1, 0:1])

            # ---- phase 1: Y0/Y1 matmuls + per-channel sum / sumsq ----
            with tc.tile_pool(name="ps1", bufs=3, space="PSUM") as ps1:
                for b in range(B):
                    y1ps = ps1.tile([K, D], fp32, tag="y1ps")
                    for h in range(2):
                        nc.tensor.matmul(
                            y1ps[:],
                            kct[:, b * 256 + h * 128 : b * 256 + h * 128 + 128],
                            w1t[:, h * D : (h + 1) * D],
                            start=(h == 0),
                            stop=(h == 1),
                        )
                    nc.scalar.copy(y1sb[:, b * D : (b + 1) * D], y1ps[:])

                    y0ps = ps1.tile([CL, D], fp32, tag="y0ps")
                    for h in range(2):
                        nc.tensor.matmul(
                            y0ps[:],
                            qt[:, b * 2 * CL + h * CL : b * 2 * CL + (h + 1) * CL],
                            w0t[:, h * D : (h + 1) * D],
                            start=(h == 0),
                            stop=(h == 1),
                        )
                    nc.scalar.copy(y0sb[:, b * D : (b + 1) * D], y0ps[:])

                    # per-2-batch channel stats on the vector engine, from
                    # the SBUF copies (vector must not touch live PSUM here)
                    if b % 2 == 1:
                        i4 = b // 2
                        c0, c1 = (b - 1) * D, (b + 1) * D
                        nc.vector.tensor_reduce(
                            s1cols[:, i4 : i4 + 1], y1sb[:, c0:c1], AX.X, OP.add
                        )
                        nc.vector.tensor_mul(sq1s[:], y1sb[:, c0:c1], y1sb[:, c0:c1])
                        nc.vector.tensor_reduce(
                            q1cols[:, i4 : i4 + 1], sq1s[:], AX.X, OP.add
                        )
                        nc.vector.tensor_reduce(
                            s0cols[:, i4 : i4 + 1], y0sb[:, c0:c1], AX.X, OP.add
                        )
                        nc.vector.tensor_mul(sq0s[:], y0sb[:, c0:c1], y0sb[:, c0:c1])
                        nc.vector.tensor_reduce(
                            q0cols[:, i4 : i4 + 1], sq0s[:], AX.X, OP.add
                        )

            # ---- phase boundary: finalize BN scale/shift ----
            # s = gamma / sqrt(var+eps);  t = beta - mean * s
            # Mostly on the scalar engine: DVE ops pay a pipeline DRAIN each,
            # which dominates this serial chain of tiny (P,1) ops.
            def bn_finalize(P, scols, qcols, gc, bc):
                ssum = sp.tile([P, 1], fp32, name=f"ssum{P}")
                nc.vector.tensor_reduce(ssum[:], scols[:, 0:8], AX.X, OP.add)
                qsum = sp.tile([P, 1], fp32, name=f"qsum{P}")
                nc.vector.tensor_reduce(qsum[:], qcols[:, 0:8], AX.X, OP.add)
                mean = sp.tile([P, 1], fp32, name=f"mean{P}")
                nc.scalar.mul(mean[:], ssum[:], 1.0 / BD)
                # ex2e = E[x^2] + eps
                ex2e = sp.tile([P, 1], fp32, name=f"ex2e{P}")
                nc.scalar.activation(
                    ex2e[:], qsum[:], AF.Identity, bias=epst[:P], scale=1.0 / BD
                )
                msq = sp.tile([P, 1], fp32, name=f"msq{P}")
                nc.scalar.square(msq[:], mean[:])
                # varp = ex2e - mean^2
                varp = sp.tile([P, 1], fp32, name=f"varp{P}")
                nc.scalar.activation(
                    varp[:], msq[:], AF.Identity, bias=ex2e[:], scale=-1.0
                )
                std = sp.tile([P, 1], fp32, name=f"std{P}")
                nc.scalar.sqrt(std[:], varp[:])
                # one Newton step to clean up the scalar-engine sqrt:
                # std' = 0.5*(std + varp/std)
                rstd = sp.tile([P, 1], fp32, name=f"rstd{P}")
                nc.vector.reciprocal(rstd[:], std[:])
                qh = sp.tile([P, 1], fp32, name=f"qh{P}")
                nc.scalar.mul(qh[:], varp[:], rstd[:])  # varp/std
                stdh = sp.tile([P, 1], fp32, name=f"stdh{P}")
                nc.scalar.mul(stdh[:], std[:], 0.5)
                std2 = sp.tile([P, 1], fp32, name=f"std2{P}")
                nc.scalar.activation(
                    std2[:], qh[:], AF.Identity, bias=stdh[:], scale=0.5
                )
                inv = sp.tile([P, 1], fp32, name=f"inv{P}")
                nc.vector.reciprocal(inv[:], std2[:])
                s_ = sp.tile([P, 1], fp32, name=f"s_{P}")
                nc.scalar.mul(s_[:], inv[:], gc[:])
                ms = sp.tile([P, 1], fp32, name=f"ms{P}")
                nc.scalar.mul(ms[:], mean[:], s_[:])
                t_ = sp.tile([P, 1], fp32, name=f"t_{P}")
                nc.scalar.activation(
                    t_[:], ms[:], AF.Identity, bias=bc[:], scale=-1.0
                )
                return s_, t_

            def bn_finalize_v(P, scols, qcols, gc, bc):
                # vector-engine variant so BN0 finalizes concurrently with
                # BN1 on the scalar engine
                ssum = sp.tile([P, 1], fp32, name=f"vssum{P}")
                nc.vector.tensor_reduce(ssum[:], scols[:, 0:8], AX.X, OP.add)
                qsum = sp.tile([P, 1], fp32, name=f"vqsum{P}")
                nc.vector.tensor_reduce(qsum[:], qcols[:, 0:8], AX.X, OP.add)
                mean = sp.tile([P, 1], fp32, name=f"vmean{P}")
                nc.vector.tensor_scalar_mul(mean[:], ssum[:], 1.0 / BD)
                ex2e = sp.tile([P, 1], fp32, name=f"vex2e{P}")
                nc.vector.tensor_scalar(
                    ex2e[:], qsum[:], 1.0 / BD, EPS, OP.mult, OP.add
                )
                msq = sp.tile([P, 1], fp32, name=f"vmsq{P}")
                nc.vector.tensor_mul(msq[:], mean[:], mean[:])
                varp = sp.tile([P, 1], fp32, name=f"vvarp{P}")
                nc.vector.tensor_sub(varp[:], ex2e[:], msq[:])
                std = sp.tile([P, 1], fp32, name=f"vstd{P}")
                nc.scalar.sqrt(std[:], varp[:])
                rstd = sp.tile([P, 1], fp32, name=f"vrstd{P}")
                nc.vector.reciprocal(rstd[:], std[:])
                q_ = sp.tile([P, 1], fp32, name=f"vq_{P}")
                nc.vector.tensor_mul(q_[:], varp[:], rstd[:])
                nc.vector.tensor_add(std[:], std[:], q_[:])
                nc.vector.tensor_scalar_mul(std[:], std[:], 0.5)
                inv = sp.tile([P, 1], fp32, name=f"vinv{P}")
                nc.vector.reciprocal(inv[:], std[:])
                s_ = sp.tile([P, 1], fp32, name=f"vs_{P}")
                nc.vector.tensor_mul(s_[:], inv[:], gc[:])
                ms = sp.tile([P, 1], fp32, name=f"vms{P}")
                nc.vector.tensor_mul(ms[:], mean[:], s_[:])
                t_ = sp.tile([P, 1], fp32, name=f"vt_{P}")
                nc.vector.tensor_sub(t_[:], bc[:], ms[:])
                return s_, t_

            s1, t1 = bn_finalize(K, s1cols, q1cols, g1c, b1c)
            s0, t0 = bn_finalize_v(CL, s0cols, q0cols, g0c, b0c)

            # Bake cmask into a per-(c,b) scale/bias so Qg = sigmoid-masked
            # comes straight off the scalar engine:
            #   masked: sigmoid(s0*y + t0);  unmasked: sigmoid(0*y - 1e30) = 0
            s0b = sp.tile([CL, B], fp32)
            nc.vector.tensor_scalar(s0b[:], cmt[:], s0[:], None, OP.mult)
            t0b = sp.tile([CL, B], fp32)
            # t0b = t0*cm + (cm-1)*1e30
            nc.vector.tensor_scalar(t0b[:], cmt[:], 1.0, 1e30, OP.subtract, OP.mult)
            tb2 = sp.tile([CL, B], fp32)
            nc.vector.tensor_scalar(tb2[:], cmt[:], t0[:], None, OP.mult)
            nc.vector.tensor_add(t0b[:], t0b[:], tb2[:])

            # ---- phase 2 ----
            # PSUM transpose-staging layout (single bank):
            #   [  0:128) sig1T h0   [128:256) sig1T h1
            #   [256:272) sig0T h0   [272:288) sig0T h1
            #   [288:304) QgT  h0    [304:320) QgT  h1
            S1T, S0T, QGT = 0, 256, 288
            with (
                tc.tile_pool(name="pst", bufs=2, space="PSUM") as pst,  # transposes
                tc.tile_pool(name="psr", bufs=2, space="PSUM") as psr,  # awm
            ):
                for b in range(B):
                    # alternate HWDGE queues so the big output DMA never
                    # head-of-line-blocks the small pipeline DMAs
                    dq = nc.sync if (b % 2 == 0) else nc.scalar
                    oq = nc.scalar if (b % 2 == 0) else nc.sync

                    yb = y1sb[:, b * D : (b + 1) * D]
                    sig1 = wp.tile([K, D], fp32, tag="sig1")
                    nc.scalar.activation(
                        sig1[:], yb, AF.Sigmoid, bias=t1[:], scale=s1[:]
                    )

                    sig0 = wp.tile([CL, D], fp32, tag="sig0")
                    nc.scalar.activation(
                        sig0[:],
                        y0sb[:, b * D : (b + 1) * D],
                        AF.Sigmoid,
                        bias=t0[:],
                        scale=s0[:],
                    )
                    qg = wp.tile([CL, D], fp32, tag="qg")
                    nc.scalar.activation(
                        qg[:],
                        y0sb[:, b * D : (b + 1) * D],
                        AF.Sigmoid,
                        bias=t0b[:, b : b + 1],
                        scale=s0b[:, b : b + 1],
                    )

                    # transpose into d-on-partitions layout
                    tps = pst.tile([128, 320], fp32, tag="tps")
                    for h in range(2):
                        nc.tensor.transpose(
                            tps[:, S1T + h * K : S1T + (h + 1) * K],
                            sig1[:, h * 128 : (h + 1) * 128],
                            iden[:, 0:128],
                        )
                        nc.tensor.transpose(
                            tps[:, S0T + h * CL : S0T + (h + 1) * CL],
                            sig0[:, h * 128 : (h + 1) * 128],
                            iden[0:CL, 0:CL],
                        )
                        nc.tensor.transpose(
                            tps[:, QGT + h * CL : QGT + (h + 1) * CL],
                            qg[:, h * 128 : (h + 1) * 128],
                            iden[0:CL, 0:CL],
                        )
                    st = wp.tile([128, 320], fp32, tag="st")
                    nc.scalar.copy(st[:], tps[:])


                    # A_t[d, k] = sig1T[d, k] * (kc*kmask)T[d, k];
                    # accum_out gives sum_k A_t = the attention-vector sum
                    at2 = wp.tile([128, 2 * K], fp32, tag="at2")
                    sA = wp.tile([128, 2], fp32, tag="sA")
                    for h in range(2):
                        nc.vector.scalar_tensor_tensor(
                            at2[:, h * K : (h + 1) * K],
                            st[:, S1T + h * K : S1T + (h + 1) * K],
                            1.0,
                            kcm[:, b * 2 * K + h * K : b * 2 * K + (h + 1) * K],
                            op0=OP.bypass,
                            op1=OP.mult,
                            accum_out=sA[:, h : h + 1],
                        )

                    # awm[c,k] = (1/D) * sum_d sig0T[d,c] * sig1T[d,k]
                    psr_t = psr.tile([CL, K], fp32, tag="psr")
                    for h in range(2):
                        nc.tensor.matmul(
                            psr_t[:],
                            st[:, S0T + h * CL : S0T + (h + 1) * CL],
                            st[:, S1T + h * K : S1T + (h + 1) * K],
                            start=(h == 0),
                            stop=(h == 1),
                        )
                    nc.scalar.mul(
                        awm_all[:, b * K : (b + 1) * K], psr_t[:], 1.0 / D
                    )

                    # attention_vector (transposed): sum_k A_t along free,
                    # then av_t[d,c] = QgT[d,c] * sumA[d], tanh(av/klen)
                    av_t = wp.tile([128, 2 * CL], fp32, tag="av_t")
                    for h in range(2):
                        nc.scalar.mul(
                            av_t[:, h * CL : (h + 1) * CL],
                            st[:, QGT + h * CL : QGT + (h + 1) * CL],
                            sA[:, h : h + 1],
                        )
                    nc.scalar.activation(
                        ores_all[:, b * 2 * CL : (b + 1) * 2 * CL],
                        av_t[:],
                        AF.Tanh,
                        bias=0.0,
                        scale=ilen[:, b : b + 1],
                    )

                    # big product, d on partitions:
                    #   big_t[d, c, k] = QgT[d, c] * A_t[d, k]
                    big = bp.tile([128, 2 * CL * K], fp32, tag="big")
                    nc.vector.tensor_tensor(
                        big[:].rearrange("p (h c k) -> p h c k", c=CL, k=K),
                        st[:, QGT : QGT + 2 * CL]
                        .rearrange("p (h c) -> p h c", c=CL)
                        .unsqueeze(3)
                        .to_broadcast([128, 2, CL, K]),
                        at2[:]
                        .rearrange("p (h k) -> p h k", k=K)
                        .unsqueeze(2)
                        .to_broadcast([128, 2, CL, K]),
                        OP.mult,
                    )
                    dq.dma_start(
                        attn_d.ap()[b].rearrange("h p c k -> p h c k"),
                        big[:].rearrange("p (h c k) -> p h c k", c=CL, k=K),
                    )

                if True:
                    nc.scalar.dma_start(
                        ores_d.ap().rearrange("b h p c -> p b h c"),
                        ores_all[:].rearrange("p (b h c) -> p b h c", h=2, c=CL),
                    )
                    nc.scalar.dma_start(
                        awm_d.ap().rearrange("b c k -> c b k"),
                        awm_all[:].rearrange("c (b k) -> c b k", k=K),
                    )

    nc.compile()
    return nc


def _get_nc():
    if "nc" not in _CACHE:
        _CACHE["nc"] = _build_nc()
    return _CACHE["nc"]


def _make_in_maps(inputs):
    q = np.ascontiguousarray(inputs["query_candidates_repr"], dtype=np.float32)
    kc = np.ascontiguousarray(inputs["key_candidates"], dtype=np.float32)
    W0 = np.asarray(inputs["W0"], dtype=np.float32)
    W1 = np.asarray(inputs["W1"], dtype=np.float32)
    g0 = np.asarray(inputs["bn0_gamma"], dtype=np.float32)
    b0 = np.asarray(inputs["bn0_beta"], dtype=np.float32)
    g1 = np.asarray(inputs["bn1_gamma"], dtype=np.float32)
    b1 = np.asarray(inputs["bn1_beta"], dtype=np.float32)
    cm = np.asarray(inputs["query_candidate_mask"]).astype(np.float32)
    km = np.asarray(inputs["key_candidate_mask"]).astype(np.float32)
    kl = np.asarray(inputs["key_candidate_len"]).astype(np.float32)

    kct = np.ascontiguousarray(
        kc.reshape(B, K, 2, 128).transpose(3, 0, 2, 1)
    )  # (128, B, 2, K)
    kcm = np.ascontiguousarray(
        (kc * km[:, :, None]).reshape(B, K, 2, 128).transpose(3, 0, 2, 1)
    )  # (128, B, 2, K), kmask folded in
    w0t = np.ascontiguousarray(W0.reshape(D, 2, 128).transpose(2, 1, 0))
    w1t = np.ascontiguousarray(W1.reshape(D, 2, 128).transpose(2, 1, 0))

    shared = dict(kct=kct, kcm=kcm, w0t=w0t, w1t=w1t)
    in_maps = []
    for r in range(NCORES):
        sl = slice(r * CL, (r + 1) * CL)
        qt = np.ascontiguousarray(
            q[:, sl, :].reshape(B, CL, 2, 128).transpose(3, 0, 2, 1)
        )
        cpk = np.zeros((128, 164), np.float32)
        cpk[:, 0] = g1
        cpk[:, 1] = b1
        cpk[:CL, 2] = g0[sl]
        cpk[:CL, 3] = b0[sl]
        cpk[:, 4 : 4 + B] = np.tile(1.0 / kl, (128, 1))
        cpk[:CL, 20 : 20 + B] = cm[:, sl].T
        cpk[:, 36:164] = np.eye(128, dtype=np.float32)
        m = dict(shared, qt=qt, cpk=cpk)
        in_maps.append(m)
    return in_maps


def run(inputs, trace=False):
    from concourse import bass_utils

    nc = _get_nc()
    in_maps = _make_in_maps(inputs)
    res = bass_utils.run_bass_kernel_spmd(
        nc, in_maps, core_ids=list(range(NCORES)), trace=trace
    )
    # device outputs are d-on-partitions (B, 2, 128, CL[, K]); restore layout
    ores_t = np.stack([res.results[r]["o_res"] for r in range(NCORES)], axis=3)
    # (B, 2, 128, NCORES, CL) -> (B, C, D)
    out_res = np.ascontiguousarray(
        ores_t.transpose(0, 3, 4, 1, 2).reshape(B, C, D)
    )
    attn_t = np.stack([res.results[r]["o_attn"] for r in range(NCORES)], axis=3)
    # (B, 2, 128, NCORES, CL, K) -> (B, C, K, D)
    attn = np.ascontiguousarray(
        attn_t.transpose(0, 3, 4, 5, 1, 2).reshape(B, C, K, D)
    )
    awm = np.concatenate([res.results[r]["o_awm"] for r in range(NCORES)], axis=1)
    return (out_res, attn, awm), res


def kernel(**inputs):
    (out_res, attn, awm), _ = run(inputs, trace=False)
    return out_res, attn, awm


# revision 52
# speedup vs baseline: 1.1100x; 1.0994x over previous
"""Trainium2 Bass kernel for nn_AttentionLayer_85383949844589.

Gated attention layer: B=16, C=K=128, D=256.
  g0 = BN0(q @ W0.T)          per-C-channel stats over (B, D)
  g1 = BN1(kc @ W1.T)         per-K-channel stats over (B, D)
  aw[b,c,k,d]   = sigmoid(g1)[b,k,d] * sigmoid(g0)[b,c,d]
  attn[b,c,k,d] = kc[b,k,d] * aw * cmask[b,c] * kmask[b,k]
  out[b,c,d]    = tanh(sum_k attn / klen[b])
  awm[b,c,k]    = mean_d aw

Sharding: the C (query-channel) axis is split across the 8 NeuronCores
(16 channels each).  BN0 stats are per-C-channel, so they are fully local
to a core; the g1/BN1 pipeline is replicated on every core (it is tiny).
No cross-core communication is needed at all.

Per core the dominant cost is writing its (B, C/8, K, D) = 32 MiB slice of
attn, i.e. the kernel is HBM-write-bound (~95 us at ~358 GB/s/core).

The big product is computed with D on the partition axis:
  big_t[d, c, k] = QgT[d, c] * A_t[d, k]
where QgT = sigmoid(g0)*cmask transposed and A_t = sigmoid(g1)T * (kc*kmask)T.
In that layout BOTH operands of the (C/8 x K) outer product are plain
free-dimension broadcast views (stride-0 free dims), so the DVE computes the
whole 4D block with two tensor_tensor ops per batch — no partition broadcast
is needed anywhere.  As a bonus the (d-partition, (c,k)-free) store has
8 KB-contiguous DRAM runs (vs 1 KB for the natural layout); the host
re-transposes the (B, 2, 128, C/8, K) device output once at the end.

Phase 1 (Y = x@W.T + per-channel sum/sumsq) runs on PE/scalar/vector with a
dep-free bf16 warm-up burst to lift the PE HAM throttle; phase 2 alternates
the two HWDGE queues so the 2 MB stores never head-of-line-block the
pipeline's small DMAs.  BN finalize runs split across scalar (BN1) and
vector (BN0) so the two serial chains overlap.
"""

import sys

sys.path.insert(0, "/opt/trn_rl_repo")

import numpy as np

B, C, K, D = 16, 128, 128, 256
NCORES = 8
CL = C // NCORES  # 16 query channels per core
EPS = 1e-5

_CACHE: dict = {}


def _build_nc():
    import concourse.tile as tile
    from concourse import bacc, mybir

    fp32 = mybir.dt.float32
    AF = mybir.ActivationFunctionType
    OP = mybir.AluOpType
    AX = mybir.AxisListType

    nc = bacc.Bacc(trn_type="TRN2", debug=False, num_devices=NCORES)

    # ---- DRAM I/O ----
    # qt[p, b, h, c]  = q[b, c_slice[c], h*128+p]
    qt_d = nc.dram_tensor("qt", [128, B, 2, CL], fp32, kind="ExternalInput")
    # kct[p, b, h, k] = kc[b, k, h*128+p]
    kct_d = nc.dram_tensor("kct", [128, B, 2, K], fp32, kind="ExternalInput")
    # kcm[p, b, h, k] = kc[b, k, h*128+p] * kmask[b, k]
    kcm_d = nc.dram_tensor("kcm", [128, B, 2, K], fp32, kind="ExternalInput")
    # wXt[p, h, o]    = WX[o, h*128+p]
    w0t_d = nc.dram_tensor("w0t", [128, 2, D], fp32, kind="ExternalInput")
    w1t_d = nc.dram_tensor("w1t", [128, 2, D], fp32, kind="ExternalInput")
    # all small constants packed into one tensor (single DMA):
    # cols [0]=g1, [1]=b1, [2]=g0(rows 0:CL), [3]=b0(rows 0:CL),
    # [4:4+B]=ilen(128,B), [20:20+B]=cmt(rows 0:CL), [36:164]=identity
    cpk_d = nc.dram_tensor("cpk", [128, 164], fp32, kind="ExternalInput")

    # transposed layouts (d on partitions); host reassembles
    ores_d = nc.dram_tensor("o_res", [B, 2, 128, CL], fp32, kind="ExternalOutput")
    attn_d = nc.dram_tensor("o_attn", [B, 2, 128, CL, K], fp32, kind="ExternalOutput")
    awm_d = nc.dram_tensor("o_awm", [B, CL, K], fp32, kind="ExternalOutput")

    BD = float(B * D)

    with tile.TileContext(nc) as tc:
        with (
            tc.tile_pool(name="const", bufs=1) as cp,
            tc.tile_pool(name="persist", bufs=1) as pp,
            tc.tile_pool(name="stats", bufs=1) as sp,
            tc.tile_pool(name="work", bufs=2) as wp,
            tc.tile_pool(name="bigout", bufs=4) as bp,
        ):
            # ---- load inputs; kct/kcm/w first so phase 1 starts ASAP ----
            kct = pp.tile([128, B * 2 * K], fp32)
            kcm = pp.tile([128, B * 2 * K], fp32)
            CH = B // 4

            def load_kct(i):
                eng = nc.sync if i % 2 == 0 else nc.scalar
                sl_d = slice(i * CH, (i + 1) * CH)
                sl_s = slice(i * CH * 2 * K, (i + 1) * CH * 2 * K)
                eng.dma_start(
                    kct[:, sl_s],
                    kct_d.ap()[:, sl_d].rearrange("p b h k -> p (b h k)"),
                )

            def load_kcm(i):
                eng = nc.sync if i % 2 == 0 else nc.scalar
                sl_d = slice(i * CH, (i + 1) * CH)
                sl_s = slice(i * CH * 2 * K, (i + 1) * CH * 2 * K)
                eng.dma_start(
                    kcm[:, sl_s],
                    kcm_d.ap()[:, sl_d].rearrange("p b h k -> p (b h k)"),
                )

            load_kct(0)
            load_kct(1)
            w1t = cp.tile([128, 2 * D], fp32)
            nc.sync.dma_start(w1t[:], w1t_d.ap().rearrange("p h o -> p (h o)"))
            w0t = cp.tile([128, 2 * D], fp32)
            nc.scalar.dma_start(w0t[:], w0t_d.ap().rearrange("p h o -> p (h o)"))
            load_kct(2)
            load_kct(3)
            qt = pp.tile([128, B * 2 * CL], fp32)
            nc.scalar.dma_start(qt[:], qt_d.ap().rearrange("p b h c -> p (b h c)"))
            cpk = cp.tile([128, 164], fp32)
            nc.sync.dma_start(cpk[:], cpk_d.ap()[:])
            for i in range(4):
                load_kcm(i)
            g1c = cpk[:, 0:1]
            b1c = cpk[:, 1:2]
            g0c = cpk[0:CL, 2:3]
            b0c = cpk[0:CL, 3:4]
            ilen = cpk[:, 4 : 4 + B]
            cmt = cpk[0:CL, 20 : 20 + B]
            iden = cpk[:, 36:164]

            y1sb = pp.tile([K, B * D], fp32)
            y0sb = pp.tile([CL, B * D], fp32)

            s1cols = sp.tile([K, B], fp32)
            q1cols = sp.tile([K, B], fp32)
            s0cols = sp.tile([CL, B], fp32)
            q0cols = sp.tile([CL, B], fp32)
            sq1s = sp.tile([K, 2 * D], fp32)
            sq0s = sp.tile([CL, 2 * D], fp32)
            epst = sp.tile([128, 1], fp32)
            nc.vector.memset(epst[:], EPS)
            ores_all = pp.tile([128, B * 2 * CL], fp32)
            awm_all = pp.tile([CL, B * K], fp32)

            # ---- PE warm-up burst ----
            # HAM starts the PE throttled (1.2 GHz) and only un-throttles
            # after a sustained-busy window.  Run dep-free bf16 matmuls while
            # the input DMAs land so phase 1 runs at 2.4 GHz.
            bf16 = mybir.dt.bfloat16
            wu_a = sp.tile([128, 128], bf16)
            nc.vector.memset(wu_a[:], 1.0)
            wu_b = sp.tile([128, 512], bf16)
            nc.vector.memset(wu_b[:], 1.0)
            with tc.tile_pool(name="pswu", bufs=1, space="PSUM") as pswu:
                wu_ps = pswu.tile([128, 512], fp32)
                NWU = 12
                for i in range(NWU):
                    nc.tensor.matmul(
                        wu_ps[:], wu_a[:], wu_b[:],
                        start=(i == 0), stop=(i == NWU - 1),
                    )
                wu_out = sp.tile([1, 1], fp32)
                nc.scalar.copy(wu_out[:], wu_ps[0:1, 0:1])

            # ---- phase 1: Y0/Y1 matmuls + per-channel sum / sumsq ----
            with tc.tile_pool(name="ps1", bufs=3, space="PSUM") as ps1:
                for b in range(B):
                    y1ps = ps1.tile([K, D], fp32, tag="y1ps")
                    for h in range(2):
                        nc.tensor.matmul(
                            y1ps[:],
                            kct[:, b * 256 + h * 128 : b * 256 + h * 128 + 128],
                            w1t[:, h * D : (h + 1) * D],
                            start=(h == 0),
                            stop=(h == 1),
                        )
                    nc.scalar.copy(y1sb[:, b * D : (b + 1) * D], y1ps[:])

                    y0ps = ps1.tile([CL, D], fp32, tag="y0ps")
                    for h in range(2):
                        nc.tensor.matmul(
                            y0ps[:],
                            qt[:, b * 2 * CL + h * CL : b * 2 * CL + (h + 1) * CL],
                            w0t[:, h * D : (h + 1) * D],
                            start=(h == 0),
                            stop=(h == 1),
                        )
                    nc.scalar.copy(y0sb[:, b * D : (b + 1) * D], y0ps[:])

                    # per-2-batch channel stats on the vector engine, from
                    # the SBUF copies (vector must not touch live PSUM here)
                    if b % 2 == 1:
                        i4 = b // 2
                        c0, c1 = (b - 1) * D, (b + 1) * D
                        nc.vector.tensor_reduce(
                            s1cols[:, i4 : i4 + 1], y1sb[:, c0:c1], AX.X, OP.add
                        )
                        nc.vector.tensor_mul(sq1s[:], y1sb[:, c0:c1], y1sb[:, c0:c1])
                        nc.vector.tensor_reduce(
                            q1cols[:, i4 : i4 + 1], sq1s[:], AX.X, OP.add
                        )
                        nc.vector.tensor_reduce(
                            s0cols[:, i4 : i4 + 1], y0sb[:, c0:c1], AX.X, OP.add
                        )
                        nc.vector.tensor_mul(sq0s[:], y0sb[:, c0:c1], y0sb[:, c0:c1])
                        nc.vector.tensor_reduce(
                            q0cols[:, i4 : i4 + 1], sq0s[:], AX.X, OP.add
                        )

            # ---- phase boundary: finalize BN scale/shift ----
            # s = gamma / sqrt(var+eps);  t = beta - mean * s
            # Mostly on the scalar engine: DVE ops pay a pipeline DRAIN each,
            # which dominates this serial chain of tiny (P,1) ops.
            def bn_finalize(P, scols, qcols, gc, bc):
                ssum = sp.tile([P, 1], fp32, name=f"ssum{P}")
                nc.vector.tensor_reduce(ssum[:], scols[:, 0:8], AX.X, OP.add)
                qsum = sp.tile([P, 1], fp32, name=f"qsum{P}")
                nc.vector.tensor_reduce(qsum[:], qcols[:, 0:8], AX.X, OP.add)
                mean = sp.tile([P, 1], fp32, name=f"mean{P}")
                nc.scalar.mul(mean[:], ssum[:], 1.0 / BD)
                # ex2e = E[x^2] + eps
                ex2e = sp.tile([P, 1], fp32, name=f"ex2e{P}")
                nc.scalar.activation(
                    ex2e[:], qsum[:], AF.Identity, bias=epst[:P], scale=1.0 / BD
                )
                msq = sp.tile([P, 1], fp32, name=f"msq{P}")
                nc.scalar.square(msq[:], mean[:])
                # varp = ex2e - mean^2
                varp = sp.tile([P, 1], fp32, name=f"varp{P}")
                nc.scalar.activation(
                    varp[:], msq[:], AF.Identity, bias=ex2e[:], scale=-1.0
                )
                std = sp.tile([P, 1], fp32, name=f"std{P}")
                nc.scalar.sqrt(std[:], varp[:])
                # one Newton step to clean up the scalar-engine sqrt:
                # std' = 0.5*(std + varp/std)
                rstd = sp.tile([P, 1], fp32, name=f"rstd{P}")
                nc.vector.reciprocal(rstd[:], std[:])
                qh = sp.tile([P, 1], fp32, name=f"qh{P}")
                nc.scalar.mul(qh[:], varp[:], rstd[:])  # varp/std
                stdh = sp.tile([P, 1], fp32, name=f"stdh{P}")
                nc.scalar.mul(stdh[:], std[:], 0.5)
                std2 = sp.tile([P, 1], fp32, name=f"std2{P}")
                nc.scalar.activation(
                    std2[:], qh[:], AF.Identity, bias=stdh[:], scale=0.5
                )
                inv = sp.tile([P, 1], fp32, name=f"inv{P}")
                nc.vector.reciprocal(inv[:], std2[:])
                s_ = sp.tile([P, 1], fp32, name=f"s_{P}")
                nc.scalar.mul(s_[:], inv[:], gc[:])
                ms = sp.tile([P, 1], fp32, name=f"ms{P}")
                nc.scalar.mul(ms[:], mean[:], s_[:])
                t_ = sp.tile([P, 1], fp32, name=f"t_{P}")
                nc.scalar.activation(
                    t_[:], ms[:], AF.Identity, bias=bc[:], scale=-1.0
                )
                return s_, t_

            def bn_finalize_v(P, scols, qcols, gc, bc):
                # vector-engine variant so BN0 finalizes concurrently with
                # BN1 on the scalar engine
                ssum = sp.tile([P, 1], fp32, name=f"vssum{P}")
                nc.vector.tensor_reduce(ssum[:], scols[:, 0:8], AX.X, OP.add)
                qsum = sp.tile([P, 1], fp32, name=f"vqsum{P}")
                nc.vector.tensor_reduce(qsum[:], qcols[:, 0:8], AX.X, OP.add)
                mean = sp.tile([P, 1], fp32, name=f"vmean{P}")
                nc.vector.tensor_scalar_mul(mean[:], ssum[:], 1.0 / BD)
                ex2e = sp.tile([P, 1], fp32, name=f"vex2e{P}")
                nc.vector.tensor_scalar(
                    ex2e[:], qsum[:], 1.0 / BD, EPS, OP.mult, OP.add
                )
                msq = sp.tile([P, 1], fp32, name=f"vmsq{P}")
                nc.vector.tensor_mul(msq[:], mean[:], mean[:])
                varp = sp.tile([P, 1], fp32, name=f"vvarp{P}")
                nc.vector.tensor_sub(varp[:], ex2e[:], msq[:])
                std = sp.tile([P, 1], fp32, name=f"vstd{P}")
                nc.scalar.sqrt(std[:], varp[:])
                rstd = sp.tile([P, 1], fp32, name=f"vrstd{P}")
                nc.vector.reciprocal(rstd[:], std[:])
                q_ = sp.tile([P, 1], fp32, name=f"vq_{P}")
                nc.vector.tensor_mul(q_[:], varp[:], rstd[:])
                nc.vector.tensor_add(std[:], std[:], q_[:])
                nc.vector.tensor_scalar_mul(std[:], std[:], 0.5)
                inv = sp.tile([P, 1], fp32, name=f"vinv{P}")
                nc.vector.reciprocal(inv[:], std[:])
                s_ = sp.tile([P, 1], fp32, name=f"vs_{P}")
                nc.vector.tensor_mul(s_[:], inv[:], gc[:])
                ms = sp.tile([P, 1], fp32, name=f"vms{P}")
                nc.vector.tensor_mul(ms[:], mean[:], s_[:])
                t_ = sp.tile([P, 1], fp32, name=f"vt_{P}")
                nc.vector.tensor_sub(t_[:], bc[:], ms[:])
                return s_, t_

            s1, t1 = bn_finalize(K, s1cols, q1cols, g1c, b1c)
            s0, t0 = bn_finalize_v(CL, s0cols, q0cols, g0c, b0c)

            # Bake cmask into a per-(c,b) scale/bias so Qg = sigmoid-masked
            # comes straight off the scalar engine:
            #   masked: sigmoid(s0*y + t0);  unmasked: sigmoid(0*y - 1e30) = 0
            s0b = sp.tile([CL, B], fp32)
            nc.vector.tensor_scalar(s0b[:], cmt[:], s0[:], None, OP.mult)
            t0b = sp.tile([CL, B], fp32)
            # t0b = t0*cm + (cm-1)*1e30
            nc.vector.tensor_scalar(t0b[:], cmt[:], 1.0, 1e30, OP.subtract, OP.mult)
            tb2 = sp.tile([CL, B], fp32)
            nc.vector.tensor_scalar(tb2[:], cmt[:], t0[:], None, OP.mult)
            nc.vector.tensor_add(t0b[:], t0b[:], tb2[:])

            # ---- phase 2 ----
            # PSUM transpose-staging layout (single bank):
            #   [  0:128) sig1T h0   [128:256) sig1T h1
            #   [256:272) sig0T h0   [272:288) sig0T h1
            #   [288:304) QgT  h0    [304:320) QgT  h1
            S1T, S0T, QGT = 0, 256, 288
            with (
                tc.tile_pool(name="pst", bufs=2, space="PSUM") as pst,  # transposes
                tc.tile_pool(name="psr", bufs=2, space="PSUM") as psr,  # awm
            ):
                for b in range(B):
                    # alternate HWDGE queues so the big output DMA never
                    # head-of-line-blocks the small pipeline DMAs
                    dq = nc.sync if (b % 2 == 0) else nc.scalar
                    oq = nc.scalar if (b % 2 == 0) else nc.sync

                    yb = y1sb[:, b * D : (b + 1) * D]
                    sig1 = wp.tile([K, D], fp32, tag="sig1")
                    nc.scalar.activation(
                        sig1[:], yb, AF.Sigmoid, bias=t1[:], scale=s1[:]
                    )

                    sig0 = wp.tile([CL, D], fp32, tag="sig0")
                    nc.scalar.activation(
                        sig0[:],
                        y0sb[:, b * D : (b + 1) * D],
                        AF.Sigmoid,
                        bias=t0[:],
                        scale=s0[:],
                    )
                    qg = wp.tile([CL, D], fp32, tag="qg")
                    nc.scalar.activation(
                        qg[:],
                        y0sb[:, b * D : (b + 1) * D],
                        AF.Sigmoid,
                        bias=t0b[:, b : b + 1],
                        scale=s0b[:, b : b + 1],
                    )

                    # transpose into d-on-partitions layout
                    tps = pst.tile([128, 320], fp32, tag="tps")
                    for h in range(2):
                        nc.tensor.transpose(
                            tps[:, S1T + h * K : S1T + (h + 1) * K],
                            sig1[:, h * 128 : (h + 1) * 128],
                            iden[:, 0:128],
                        )
                        nc.tensor.transpose(
                            tps[:, S0T + h * CL : S0T + (h + 1) * CL],
                            sig0[:, h * 128 : (h + 1) * 128],
                            iden[0:CL, 0:CL],
                        )
                        nc.tensor.transpose(
                            tps[:, QGT + h * CL : QGT + (h + 1) * CL],
                            qg[:, h * 128 : (h + 1) * 128],
                            iden[0:CL, 0:CL],
                        )
                    st = wp.tile([128, 320], fp32, tag="st")
                    nc.scalar.copy(st[:], tps[:])


                    # A_t[d, k] = sig1T[d, k] * (kc*kmask)T[d, k];
                    # accum_out gives sum_k A_t = the attention-vector sum
                    at2 = wp.tile([128, 2 * K], fp32, tag="at2")
                    sA = wp.tile([128, 2], fp32, tag="sA")
                    for h in range(2):
                        nc.vector.scalar_tensor_tensor(
                            at2[:, h * K : (h + 1) * K],
                            st[:, S1T + h * K : S1T + (h + 1) * K],
                            1.0,
                            kcm[:, b * 2 * K + h * K : b * 2 * K + (h + 1) * K],
                            op0=OP.bypass,
                            op1=OP.mult,
                            accum_out=sA[:, h : h + 1],
                        )

                    # awm[c,k] = (1/D) * sum_d sig0T[d,c] * sig1T[d,k]
                    psr_t = psr.tile([CL, K], fp32, tag="psr")
                    for h in range(2):
                        nc.tensor.matmul(
                            psr_t[:],
                            st[:, S0T + h * CL : S0T + (h + 1) * CL],
                            st[:, S1T + h * K : S1T + (h + 1) * K],
                            start=(h == 0),
                            stop=(h == 1),
                        )
                    nc.scalar.mul(
                        awm_all[:, b * K : (b + 1) * K], psr_t[:], 1.0 / D
                    )

                    # attention_vector (transposed): sum_k A_t along free,
                    # then av_t[d,c] = QgT[d,c] * sumA[d], tanh(av/klen)
                    av_t = wp.tile([128, 2 * CL], fp32, tag="av_t")
                    for h in range(2):
                        nc.scalar.mul(
                            av_t[:, h * CL : (h + 1) * CL],
                            st[:, QGT + h * CL : QGT + (h + 1) * CL],
                            sA[:, h : h + 1],
                        )
                    nc.scalar.activation(
                        ores_all[:, b * 2 * CL : (b + 1) * 2 * CL],
                        av_t[:],
                        AF.Tanh,
                        bias=0.0,
                        scale=ilen[:, b : b + 1],
                    )

                    # big product, d on partitions:
                    #   big_t[d, c, k] = QgT[d, c] * A_t[d, k]
                    big = bp.tile([128, 2 * CL * K], fp32, tag="big")
                    nc.vector.tensor_tensor(
                        big[:].rearrange("p (h c k) -> p h c k", c=CL, k=K),
                        st[:, QGT : QGT + 2 * CL]
                        .rearrange("p (h c) -> p h c", c=CL)
                        .unsqueeze(3)
                        .to_broadcast([128, 2, CL, K]),
                        at2[:]
                        .rearrange("p (h k) -> p h k", k=K)
                        .unsqueeze(2)
                        .to_broadcast([128, 2, CL, K]),
                        OP.mult,
                    )
                    dq.dma_start(
                        attn_d.ap()[b].rearrange("h p c k -> p h c k"),
                        big[:].rearrange("p (h c k) -> p h c k", c=CL, k=K),
                    )

                if True:
                    nc.scalar.dma_start(
                        ores_d.ap().rearrange("b h p c -> p b h c"),
                        ores_all[:].rearrange("p (b h c) -> p b h c", h=2, c=CL),
                    )
                    nc.scalar.dma_start(
                        awm_d.ap().rearrange("b c k -> c b k"),
                        awm_all[:].rearrange("c (b k) -> c b k", k=K),
                    )

    nc.compile()
    return nc


def _get_nc():
    if "nc" not in _CACHE:
        _CACHE["nc"] = _build_nc()
    return _CACHE["nc"]


def _make_in_maps(inputs):
    q = np.ascontiguousarray(inputs["query_candidates_repr"], dtype=np.float32)
    kc = np.ascontiguousarray(inputs["key_candidates"], dtype=np.float32)
    W0 = np.asarray(inputs["W0"], dtype=np.float32)
    W1 = np.asarray(inputs["W1"], dtype=np.float32)
    g0 = np.asarray(inputs["bn0_gamma"], dtype=np.float32)
    b0 = np.asarray(inputs["bn0_beta"], dtype=np.float32)
    g1 = np.asarray(inputs["bn1_gamma"], dtype=np.float32)
    b1 = np.asarray(inputs["bn1_beta"], dtype=np.float32)
    cm = np.asarray(inputs["query_candidate_mask"]).astype(np.float32)
    km = np.asarray(inputs["key_candidate_mask"]).astype(np.float32)
    kl = np.asarray(inputs["key_candidate_len"]).astype(np.float32)

    kct = np.ascontiguousarray(
        kc.reshape(B, K, 2, 128).transpose(3, 0, 2, 1)
    )  # (128, B, 2, K)
    kcm = np.ascontiguousarray(
        (kc * km[:, :, None]).reshape(B, K, 2, 128).transpose(3, 0, 2, 1)
    )  # (128, B, 2, K), kmask folded in
    w0t = np.ascontiguousarray(W0.reshape(D, 2, 128).transpose(2, 1, 0))
    w1t = np.ascontiguousarray(W1.reshape(D, 2, 128).transpose(2, 1, 0))

    shared = dict(kct=kct, kcm=kcm, w0t=w0t, w1t=w1t)
    in_maps = []
    for r in range(NCORES):
        sl = slice(r * CL, (r + 1) * CL)
        qt = np.ascontiguousarray(
            q[:, sl, :].reshape(B, CL, 2, 128).transpose(3, 0, 2, 1)
        )
        cpk = np.zeros((128, 164), np.float32)
        cpk[:, 0] = g1
        cpk[:, 1] = b1
        cpk[:CL, 2] = g0[sl]
        cpk[:CL, 3] = b0[sl]
        cpk[:, 4 : 4 + B] = np.tile(1.0 / kl, (128, 1))
        cpk[:CL, 20 : 20 + B] = cm[:, sl].T
        cpk[:, 36:164] = np.eye(128, dtype=np.float32)
        m = dict(shared, qt=qt, cpk=cpk)
        in_maps.append(m)
    return in_maps


def run(inputs, trace=False):
    from concourse import bass_utils

    nc = _get_nc()
    in_maps = _make_in_maps(inputs)
    res = bass_utils.run_bass_kernel_spmd(
        nc, in_maps, core_ids=list(range(NCORES)), trace=trace
    )
    # device outputs are d-on-partitions (B, 2, 128, CL[, K]); restore layout
    ores_t = np.stack([res.results[r]["o_res"] for r in range(NCORES)], axis=3)
    # (B, 2, 128, NCORES, CL) -> (B, C, D)
    out_res = np.ascontiguousarray(
        ores_t.transpose(0, 3, 4, 1, 2).reshape(B, C, D)
    )
    attn_t = np.stack([res.results[r]["o_attn"] for r in range(NCORES)], axis=3)
    # (B, 2, 128, NCORES, CL, K) -> (B, C, K, D)
    attn = np.ascontiguousarray(
        attn_t.transpose(0, 3, 4, 5, 1, 2).reshape(B, C, K, D)
    )
    awm = np.concatenate([res.results[r]["o_awm"] for r in range(NCORES)], axis=1)
    return (out_res, attn, awm), res


def kernel(**inputs):
    (out_res, attn, awm), _ = run(inputs, trace=False)
    return out_res, attn, awm


# revision 53
# speedup vs baseline: 1.1201x; 1.0091x over previous
"""Trainium2 Bass kernel for nn_AttentionLayer_85383949844589.

Gated attention layer: B=16, C=K=128, D=256.
  g0 = BN0(q @ W0.T)          per-C-channel stats over (B, D)
  g1 = BN1(kc @ W1.T)         per-K-channel stats over (B, D)
  aw[b,c,k,d]   = sigmoid(g1)[b,k,d] * sigmoid(g0)[b,c,d]
  attn[b,c,k,d] = kc[b,k,d] * aw * cmask[b,c] * kmask[b,k]
  out[b,c,d]    = tanh(sum_k attn / klen[b])
  awm[b,c,k]    = mean_d aw

Sharding: the C (query-channel) axis is split across the 8 NeuronCores
(16 channels each).  BN0 stats are per-C-channel, so they are fully local
to a core; the g1/BN1 pipeline is replicated on every core (it is tiny).
No cross-core communication is needed at all.

Per core the dominant cost is writing its (B, C/8, K, D) = 32 MiB slice of
attn, i.e. the kernel is HBM-write-bound (~95 us at ~358 GB/s/core).

The big product is computed with D on the partition axis:
  big_t[d, c, k] = QgT[d, c] * A_t[d, k]
where QgT = sigmoid(g0)*cmask transposed and A_t = sigmoid(g1)T * (kc*kmask)T.
In that layout BOTH operands of the (C/8 x K) outer product are plain
free-dimension broadcast views (stride-0 free dims), so the DVE computes the
whole 4D block with two tensor_tensor ops per batch — no partition broadcast
is needed anywhere.  As a bonus the (d-partition, (c,k)-free) store has
8 KB-contiguous DRAM runs (vs 1 KB for the natural layout); the host
re-transposes the (B, 2, 128, C/8, K) device output once at the end.

Phase 1 (Y = x@W.T + per-channel sum/sumsq) runs on PE/scalar/vector with a
dep-free bf16 warm-up burst to lift the PE HAM throttle; phase 2 alternates
the two HWDGE queues so the 2 MB stores never head-of-line-block the
pipeline's small DMAs.  BN finalize runs split across scalar (BN1) and
vector (BN0) so the two serial chains overlap.
"""

import sys

sys.path.insert(0, "/opt/trn_rl_repo")

import numpy as np

B, C, K, D = 16, 128, 128, 256
NCORES = 8
CL = C // NCORES  # 16 query channels per core
EPS = 1e-5

_CACHE: dict = {}


def _build_nc():
    import concourse.tile as tile
    from concourse import bacc, mybir

    fp32 = mybir.dt.float32
    AF = mybir.ActivationFunctionType
    OP = mybir.AluOpType
    AX = mybir.AxisListType

    nc = bacc.Bacc(trn_type="TRN2", debug=False, num_devices=NCORES)

    # ---- DRAM I/O ----
    # qt[p, b, h, c]  = q[b, c_slice[c], h*128+p]
    qt_d = nc.dram_tensor("qt", [128, B, 2, CL], fp32, kind="ExternalInput")
    # kct[p, b, h, k] = kc[b, k, h*128+p]
    kct_d = nc.dram_tensor("kct", [128, B, 2, K], fp32, kind="ExternalInput")
    # kcm[p, b, h, k] = kc[b, k, h*128+p] * kmask[b, k]
    kcm_d = nc.dram_tensor("kcm", [128, B, 2, K], fp32, kind="ExternalInput")
    # wXt[p, h, o]    = WX[o, h*128+p]
    w0t_d = nc.dram_tensor("w0t", [128, 2, D], fp32, kind="ExternalInput")
    w1t_d = nc.dram_tensor("w1t", [128, 2, D], fp32, kind="ExternalInput")
    # all small constants packed into one tensor (single DMA):
    # cols [0]=g1, [1]=b1, [2]=g0(rows 0:CL), [3]=b0(rows 0:CL),
    # [4:4+B]=ilen(128,B), [20:20+B]=cmt(rows 0:CL), [36:164]=identity
    cpk_d = nc.dram_tensor("cpk", [128, 164], fp32, kind="ExternalInput")

    # transposed layouts (d on partitions); host reassembles
    ores_d = nc.dram_tensor("o_res", [B, 2, 128, CL], fp32, kind="ExternalOutput")
    attn_d = nc.dram_tensor("o_attn", [B, 2, 128, CL, K], fp32, kind="ExternalOutput")
    awm_d = nc.dram_tensor("o_awm", [B, CL, K], fp32, kind="ExternalOutput")

    BD = float(B * D)

    with tile.TileContext(nc) as tc:
        with (
            tc.tile_pool(name="const", bufs=1) as cp,
            tc.tile_pool(name="persist", bufs=1) as pp,
            tc.tile_pool(name="stats", bufs=1) as sp,
            tc.tile_pool(name="work", bufs=2) as wp,
            tc.tile_pool(name="bigout", bufs=4) as bp,
        ):
            # ---- load inputs; kct/kcm/w first so phase 1 starts ASAP ----
            kct = pp.tile([128, B * 2 * K], fp32)
            kcm = pp.tile([128, B * 2 * K], fp32)
            CH = B // 4

            def load_kct(i):
                eng = nc.sync if i % 2 == 0 else nc.scalar
                sl_d = slice(i * CH, (i + 1) * CH)
                sl_s = slice(i * CH * 2 * K, (i + 1) * CH * 2 * K)
                eng.dma_start(
                    kct[:, sl_s],
                    kct_d.ap()[:, sl_d].rearrange("p b h k -> p (b h k)"),
                )

            def load_kcm(i):
                eng = nc.sync if i % 2 == 0 else nc.scalar
                sl_d = slice(i * CH, (i + 1) * CH)
                sl_s = slice(i * CH * 2 * K, (i + 1) * CH * 2 * K)
                eng.dma_start(
                    kcm[:, sl_s],
                    kcm_d.ap()[:, sl_d].rearrange("p b h k -> p (b h k)"),
                )

            load_kct(0)
            load_kct(1)
            w1t = cp.tile([128, 2 * D], fp32)
            nc.sync.dma_start(w1t[:], w1t_d.ap().rearrange("p h o -> p (h o)"))
            w0t = cp.tile([128, 2 * D], fp32)
            nc.scalar.dma_start(w0t[:], w0t_d.ap().rearrange("p h o -> p (h o)"))
            load_kct(2)
            load_kct(3)
            qt = pp.tile([128, B * 2 * CL], fp32)
            nc.scalar.dma_start(qt[:], qt_d.ap().rearrange("p b h c -> p (b h c)"))
            cpk = cp.tile([128, 164], fp32)
            nc.sync.dma_start(cpk[:], cpk_d.ap()[:])
            for i in range(4):
                load_kcm(i)
            g1c = cpk[:, 0:1]
            b1c = cpk[:, 1:2]
            g0c = cpk[0:CL, 2:3]
            b0c = cpk[0:CL, 3:4]
            ilen = cpk[:, 4 : 4 + B]
            cmt = cpk[0:CL, 20 : 20 + B]
            iden = cpk[:, 36:164]

            y1sb = pp.tile([K, B * D], fp32)
            y0sb = pp.tile([CL, B * D], fp32)

            s1cols = sp.tile([K, B], fp32)
            q1cols = sp.tile([K, B], fp32)
            s0cols = sp.tile([CL, B], fp32)
            q0cols = sp.tile([CL, B], fp32)
            sq1s = sp.tile([K, 2 * D], fp32)
            sq0s = sp.tile([CL, 2 * D], fp32)
            epst = sp.tile([128, 1], fp32)
            nc.vector.memset(epst[:], EPS)
            ores_all = pp.tile([128, B * 2 * CL], fp32)
            awm_all = pp.tile([CL, B * K], fp32)

            # ---- PE warm-up burst ----
            # HAM starts the PE throttled (1.2 GHz) and only un-throttles
            # after a sustained-busy window.  Run dep-free bf16 matmuls while
            # the input DMAs land so phase 1 runs at 2.4 GHz.
            bf16 = mybir.dt.bfloat16
            wu_a = sp.tile([128, 128], bf16)
            nc.vector.memset(wu_a[:], 1.0)
            wu_b = sp.tile([128, 512], bf16)
            nc.vector.memset(wu_b[:], 1.0)
            # ---- phase 1: Y0/Y1 matmuls + per-channel sum / sumsq ----
            with tc.tile_pool(name="ps1", bufs=3, space="PSUM") as ps1:
                # warm-up shares the pool so phase 1 isn't serialized behind
                # a pool close; sized to end right as the first inputs land
                wu_ps = ps1.tile([128, 512], fp32, tag="wu", bufs=1)
                NWU = 16
                for i in range(NWU):
                    nc.tensor.matmul(
                        wu_ps[:], wu_a[:], wu_b[:],
                        start=(i == 0), stop=(i == NWU - 1),
                    )
                wu_out = sp.tile([1, 1], fp32)
                nc.scalar.copy(wu_out[:], wu_ps[0:1, 0:1])

                for b in range(B):
                    y1ps = ps1.tile([K, D], fp32, tag="y1ps")
                    for h in range(2):
                        nc.tensor.matmul(
                            y1ps[:],
                            kct[:, b * 256 + h * 128 : b * 256 + h * 128 + 128],
                            w1t[:, h * D : (h + 1) * D],
                            start=(h == 0),
                            stop=(h == 1),
                        )
                    nc.scalar.copy(y1sb[:, b * D : (b + 1) * D], y1ps[:])

                    y0ps = ps1.tile([CL, D], fp32, tag="y0ps")
                    for h in range(2):
                        nc.tensor.matmul(
                            y0ps[:],
                            qt[:, b * 2 * CL + h * CL : b * 2 * CL + (h + 1) * CL],
                            w0t[:, h * D : (h + 1) * D],
                            start=(h == 0),
                            stop=(h == 1),
                        )
                    nc.scalar.copy(y0sb[:, b * D : (b + 1) * D], y0ps[:])

                    # per-2-batch channel stats on the vector engine, from
                    # the SBUF copies (vector must not touch live PSUM here)
                    if b % 2 == 1:
                        i4 = b // 2
                        c0, c1 = (b - 1) * D, (b + 1) * D
                        nc.vector.tensor_reduce(
                            s1cols[:, i4 : i4 + 1], y1sb[:, c0:c1], AX.X, OP.add
                        )
                        nc.vector.tensor_mul(sq1s[:], y1sb[:, c0:c1], y1sb[:, c0:c1])
                        nc.vector.tensor_reduce(
                            q1cols[:, i4 : i4 + 1], sq1s[:], AX.X, OP.add
                        )
                        nc.vector.tensor_reduce(
                            s0cols[:, i4 : i4 + 1], y0sb[:, c0:c1], AX.X, OP.add
                        )
                        nc.vector.tensor_mul(sq0s[:], y0sb[:, c0:c1], y0sb[:, c0:c1])
                        nc.vector.tensor_reduce(
                            q0cols[:, i4 : i4 + 1], sq0s[:], AX.X, OP.add
                        )

            # ---- phase boundary: finalize BN scale/shift ----
            # s = gamma / sqrt(var+eps);  t = beta - mean * s
            # Mostly on the scalar engine: DVE ops pay a pipeline DRAIN each,
            # which dominates this serial chain of tiny (P,1) ops.
            def bn_finalize(P, scols, qcols, gc, bc):
                ssum = sp.tile([P, 1], fp32, name=f"ssum{P}")
                nc.vector.tensor_reduce(ssum[:], scols[:, 0:8], AX.X, OP.add)
                qsum = sp.tile([P, 1], fp32, name=f"qsum{P}")
                nc.vector.tensor_reduce(qsum[:], qcols[:, 0:8], AX.X, OP.add)
                mean = sp.tile([P, 1], fp32, name=f"mean{P}")
                nc.scalar.mul(mean[:], ssum[:], 1.0 / BD)
                # ex2e = E[x^2] + eps
                ex2e = sp.tile([P, 1], fp32, name=f"ex2e{P}")
                nc.scalar.activation(
                    ex2e[:], qsum[:], AF.Identity, bias=epst[:P], scale=1.0 / BD
                )
                msq = sp.tile([P, 1], fp32, name=f"msq{P}")
                nc.scalar.square(msq[:], mean[:])
                # varp = ex2e - mean^2
                varp = sp.tile([P, 1], fp32, name=f"varp{P}")
                nc.scalar.activation(
                    varp[:], msq[:], AF.Identity, bias=ex2e[:], scale=-1.0
                )
                std = sp.tile([P, 1], fp32, name=f"std{P}")
                nc.scalar.sqrt(std[:], varp[:])
                # one Newton step to clean up the scalar-engine sqrt:
                # std' = 0.5*(std + varp/std)
                rstd = sp.tile([P, 1], fp32, name=f"rstd{P}")
                nc.vector.reciprocal(rstd[:], std[:])
                qh = sp.tile([P, 1], fp32, name=f"qh{P}")
                nc.scalar.mul(qh[:], varp[:], rstd[:])  # varp/std
                stdh = sp.tile([P, 1], fp32, name=f"stdh{P}")
                nc.scalar.mul(stdh[:], std[:], 0.5)
                std2 = sp.tile([P, 1], fp32, name=f"std2{P}")
                nc.scalar.activation(
                    std2[:], qh[:], AF.Identity, bias=stdh[:], scale=0.5
                )
                inv = sp.tile([P, 1], fp32, name=f"inv{P}")
                nc.vector.reciprocal(inv[:], std2[:])
                s_ = sp.tile([P, 1], fp32, name=f"s_{P}")
                nc.scalar.mul(s_[:], inv[:], gc[:])
                ms = sp.tile([P, 1], fp32, name=f"ms{P}")
                nc.scalar.mul(ms[:], mean[:], s_[:])
                t_ = sp.tile([P, 1], fp32, name=f"t_{P}")
                nc.scalar.activation(
                    t_[:], ms[:], AF.Identity, bias=bc[:], scale=-1.0
                )
                return s_, t_

                # bridge burst: keeps the PE HAM lit across the BN boundary
                # (the real transposes are data-gated for ~6 us anyway)
                wu2_ps = ps1.tile([128, 512], fp32, tag="wu", bufs=1)
                for i in range(8):
                    nc.tensor.matmul(
                        wu2_ps[:], wu_a[:], wu_b[:],
                        start=(i == 0), stop=(i == 7),
                    )
                wu2_out = sp.tile([1, 1], fp32)
                nc.scalar.copy(wu2_out[:], wu2_ps[0:1, 0:1])

            def bn_finalize_v(P, scols, qcols, gc, bc):
                # vector-engine variant so BN0 finalizes concurrently with
                # BN1 on the scalar engine
                ssum = sp.tile([P, 1], fp32, name=f"vssum{P}")
                nc.vector.tensor_reduce(ssum[:], scols[:, 0:8], AX.X, OP.add)
                qsum = sp.tile([P, 1], fp32, name=f"vqsum{P}")
                nc.vector.tensor_reduce(qsum[:], qcols[:, 0:8], AX.X, OP.add)
                mean = sp.tile([P, 1], fp32, name=f"vmean{P}")
                nc.vector.tensor_scalar_mul(mean[:], ssum[:], 1.0 / BD)
                ex2e = sp.tile([P, 1], fp32, name=f"vex2e{P}")
                nc.vector.tensor_scalar(
                    ex2e[:], qsum[:], 1.0 / BD, EPS, OP.mult, OP.add
                )
                msq = sp.tile([P, 1], fp32, name=f"vmsq{P}")
                nc.vector.tensor_mul(msq[:], mean[:], mean[:])
                varp = sp.tile([P, 1], fp32, name=f"vvarp{P}")
                nc.vector.tensor_sub(varp[:], ex2e[:], msq[:])
                std = sp.tile([P, 1], fp32, name=f"vstd{P}")
                nc.scalar.sqrt(std[:], varp[:])
                rstd = sp.tile([P, 1], fp32, name=f"vrstd{P}")
                nc.vector.reciprocal(rstd[:], std[:])
                q_ = sp.tile([P, 1], fp32, name=f"vq_{P}")
                nc.vector.tensor_mul(q_[:], varp[:], rstd[:])
                nc.vector.tensor_add(std[:], std[:], q_[:])
                nc.vector.tensor_scalar_mul(std[:], std[:], 0.5)
                inv = sp.tile([P, 1], fp32, name=f"vinv{P}")
                nc.vector.reciprocal(inv[:], std[:])
                s_ = sp.tile([P, 1], fp32, name=f"vs_{P}")
                nc.vector.tensor_mul(s_[:], inv[:], gc[:])
                ms = sp.tile([P, 1], fp32, name=f"vms{P}")
                nc.vector.tensor_mul(ms[:], mean[:], s_[:])
                t_ = sp.tile([P, 1], fp32, name=f"vt_{P}")
                nc.vector.tensor_sub(t_[:], bc[:], ms[:])
                return s_, t_

            s1, t1 = bn_finalize(K, s1cols, q1cols, g1c, b1c)
            s0, t0 = bn_finalize_v(CL, s0cols, q0cols, g0c, b0c)

            # Bake cmask into a per-(c,b) scale/bias so Qg = sigmoid-masked
            # comes straight off the scalar engine:
            #   masked: sigmoid(s0*y + t0);  unmasked: sigmoid(0*y - 1e30) = 0
            s0b = sp.tile([CL, B], fp32)
            nc.vector.tensor_scalar(s0b[:], cmt[:], s0[:], None, OP.mult)
            t0b = sp.tile([CL, B], fp32)
            # t0b = t0*cm + (cm-1)*1e30
            nc.vector.tensor_scalar(t0b[:], cmt[:], 1.0, 1e30, OP.subtract, OP.mult)
            tb2 = sp.tile([CL, B], fp32)
            nc.vector.tensor_scalar(tb2[:], cmt[:], t0[:], None, OP.mult)
            nc.vector.tensor_add(t0b[:], t0b[:], tb2[:])

            # ---- phase 2 ----
            # PSUM transpose-staging layout (single bank):
            #   [  0:128) sig1T h0   [128:256) sig1T h1
            #   [256:272) sig0T h0   [272:288) sig0T h1
            #   [288:304) QgT  h0    [304:320) QgT  h1
            S1T, S0T, QGT = 0, 256, 288
            with (
                tc.tile_pool(name="pst", bufs=2, space="PSUM") as pst,  # transposes
                tc.tile_pool(name="psr", bufs=2, space="PSUM") as psr,  # awm
            ):
                for b in range(B):
                    # alternate HWDGE queues so the big output DMA never
                    # head-of-line-blocks the small pipeline DMAs
                    dq = nc.sync if (b % 2 == 0) else nc.scalar
                    oq = nc.scalar if (b % 2 == 0) else nc.sync

                    yb = y1sb[:, b * D : (b + 1) * D]
                    sig1 = wp.tile([K, D], fp32, tag="sig1")
                    nc.scalar.activation(
                        sig1[:], yb, AF.Sigmoid, bias=t1[:], scale=s1[:]
                    )

                    sig0 = wp.tile([CL, D], fp32, tag="sig0")
                    nc.scalar.activation(
                        sig0[:],
                        y0sb[:, b * D : (b + 1) * D],
                        AF.Sigmoid,
                        bias=t0[:],
                        scale=s0[:],
                    )
                    qg = wp.tile([CL, D], fp32, tag="qg")
                    nc.scalar.activation(
                        qg[:],
                        y0sb[:, b * D : (b + 1) * D],
                        AF.Sigmoid,
                        bias=t0b[:, b : b + 1],
                        scale=s0b[:, b : b + 1],
                    )

                    # transpose into d-on-partitions layout
                    tps = pst.tile([128, 320], fp32, tag="tps")
                    for h in range(2):
                        nc.tensor.transpose(
                            tps[:, S1T + h * K : S1T + (h + 1) * K],
                            sig1[:, h * 128 : (h + 1) * 128],
                            iden[:, 0:128],
                        )
                        nc.tensor.transpose(
                            tps[:, S0T + h * CL : S0T + (h + 1) * CL],
                            sig0[:, h * 128 : (h + 1) * 128],
                            iden[0:CL, 0:CL],
                        )
                        nc.tensor.transpose(
                            tps[:, QGT + h * CL : QGT + (h + 1) * CL],
                            qg[:, h * 128 : (h + 1) * 128],
                            iden[0:CL, 0:CL],
                        )
                    st = wp.tile([128, 320], fp32, tag="st")
                    nc.scalar.copy(st[:], tps[:])


                    # A_t[d, k] = sig1T[d, k] * (kc*kmask)T[d, k];
                    # accum_out gives sum_k A_t = the attention-vector sum
                    at2 = wp.tile([128, 2 * K], fp32, tag="at2")
                    sA = wp.tile([128, 2], fp32, tag="sA")
                    for h in range(2):
                        nc.vector.scalar_tensor_tensor(
                            at2[:, h * K : (h + 1) * K],
                            st[:, S1T + h * K : S1T + (h + 1) * K],
                            1.0,
                            kcm[:, b * 2 * K + h * K : b * 2 * K + (h + 1) * K],
                            op0=OP.bypass,
                            op1=OP.mult,
                            accum_out=sA[:, h : h + 1],
                        )

                    # awm[c,k] = (1/D) * sum_d sig0T[d,c] * sig1T[d,k]
                    psr_t = psr.tile([CL, K], fp32, tag="psr")
                    for h in range(2):
                        nc.tensor.matmul(
                            psr_t[:],
                            st[:, S0T + h * CL : S0T + (h + 1) * CL],
                            st[:, S1T + h * K : S1T + (h + 1) * K],
                            start=(h == 0),
                            stop=(h == 1),
                        )
                    nc.scalar.mul(
                        awm_all[:, b * K : (b + 1) * K], psr_t[:], 1.0 / D
                    )

                    # attention_vector (transposed): sum_k A_t along free,
                    # then av_t[d,c] = QgT[d,c] * sumA[d], tanh(av/klen)
                    av_t = wp.tile([128, 2 * CL], fp32, tag="av_t")
                    for h in range(2):
                        nc.scalar.mul(
                            av_t[:, h * CL : (h + 1) * CL],
                            st[:, QGT + h * CL : QGT + (h + 1) * CL],
                            sA[:, h : h + 1],
                        )
                    nc.scalar.activation(
                        ores_all[:, b * 2 * CL : (b + 1) * 2 * CL],
                        av_t[:],
                        AF.Tanh,
                        bias=0.0,
                        scale=ilen[:, b : b + 1],
                    )

                    # big product, d on partitions:
                    #   big_t[d, c, k] = QgT[d, c] * A_t[d, k]
                    big = bp.tile([128, 2 * CL * K], fp32, tag="big")
                    nc.vector.tensor_tensor(
                        big[:].rearrange("p (h c k) -> p h c k", c=CL, k=K),
                        st[:, QGT : QGT + 2 * CL]
                        .rearrange("p (h c) -> p h c", c=CL)
                        .unsqueeze(3)
                        .to_broadcast([128, 2, CL, K]),
                        at2[:]
                        .rearrange("p (h k) -> p h k", k=K)
                        .unsqueeze(2)
                        .to_broadcast([128, 2, CL, K]),
                        OP.mult,
                    )
                    dq.dma_start(
                        attn_d.ap()[b].rearrange("h p c k -> p h c k"),
                        big[:].rearrange("p (h c k) -> p h c k", c=CL, k=K),
                    )

                if True:
                    nc.scalar.dma_start(
                        ores_d.ap().rearrange("b h p c -> p b h c"),
                        ores_all[:].rearrange("p (b h c) -> p b h c", h=2, c=CL),
                    )
                    nc.scalar.dma_start(
                        awm_d.ap().rearrange("b c k -> c b k"),
                        awm_all[:].rearrange("c (b k) -> c b k", k=K),
                    )

    nc.compile()
    return nc


def _get_nc():
    if "nc" not in _CACHE:
        _CACHE["nc"] = _build_nc()
    return _CACHE["nc"]


def _make_in_maps(inputs):
    q = np.ascontiguousarray(inputs["query_candidates_repr"], dtype=np.float32)
    kc = np.ascontiguousarray(inputs["key_candidates"], dtype=np.float32)
    W0 = np.asarray(inputs["W0"], dtype=np.float32)
    W1 = np.asarray(inputs["W1"], dtype=np.float32)
    g0 = np.asarray(inputs["bn0_gamma"], dtype=np.float32)
    b0 = np.asarray(inputs["bn0_beta"], dtype=np.float32)
    g1 = np.asarray(inputs["bn1_gamma"], dtype=np.float32)
    b1 = np.asarray(inputs["bn1_beta"], dtype=np.float32)
    cm = np.asarray(inputs["query_candidate_mask"]).astype(np.float32)
    km = np.asarray(inputs["key_candidate_mask"]).astype(np.float32)
    kl = np.asarray(inputs["key_candidate_len"]).astype(np.float32)

    kct = np.ascontiguousarray(
        kc.reshape(B, K, 2, 128).transpose(3, 0, 2, 1)
    )  # (128, B, 2, K)
    kcm = np.ascontiguousarray(
        (kc * km[:, :, None]).reshape(B, K, 2, 128).transpose(3, 0, 2, 1)
    )  # (128, B, 2, K), kmask folded in
    w0t = np.ascontiguousarray(W0.reshape(D, 2, 128).transpose(2, 1, 0))
    w1t = np.ascontiguousarray(W1.reshape(D, 2, 128).transpose(2, 1, 0))

    shared = dict(kct=kct, kcm=kcm, w0t=w0t, w1t=w1t)
    in_maps = []
    for r in range(NCORES):
        sl = slice(r * CL, (r + 1) * CL)
        qt = np.ascontiguousarray(
            q[:, sl, :].reshape(B, CL, 2, 128).transpose(3, 0, 2, 1)
        )
        cpk = np.zeros((128, 164), np.float32)
        cpk[:, 0] = g1
        cpk[:, 1] = b1
        cpk[:CL, 2] = g0[sl]
        cpk[:CL, 3] = b0[sl]
        cpk[:, 4 : 4 + B] = np.tile(1.0 / kl, (128, 1))
        cpk[:CL, 20 : 20 + B] = cm[:, sl].T
        cpk[:, 36:164] = np.eye(128, dtype=np.float32)
        m = dict(shared, qt=qt, cpk=cpk)
        in_maps.append(m)
    return in_maps


def run(inputs, trace=False):
    from concourse import bass_utils

    nc = _get_nc()
    in_maps = _make_in_maps(inputs)
    res = bass_utils.run_bass_kernel_spmd(
        nc, in_maps, core_ids=list(range(NCORES)), trace=trace
    )
    # device outputs are d-on-partitions (B, 2, 128, CL[, K]); restore layout
    ores_t = np.stack([res.results[r]["o_res"] for r in range(NCORES)], axis=3)
    # (B, 2, 128, NCORES, CL) -> (B, C, D)
    out_res = np.ascontiguousarray(
        ores_t.transpose(0, 3, 4, 1, 2).reshape(B, C, D)
    )
    attn_t = np.stack([res.results[r]["o_attn"] for r in range(NCORES)], axis=3)
    # (B, 2, 128, NCORES, CL, K) -> (B, C, K, D)
    attn = np.ascontiguousarray(
        attn_t.transpose(0, 3, 4, 5, 1, 2).reshape(B, C, K, D)
    )
    awm = np.concatenate([res.results[r]["o_awm"] for r in range(NCORES)], axis=1)
    return (out_res, attn, awm), res


def kernel(**inputs):
    (out_res, attn, awm), _ = run(inputs, trace=False)
    return out_res, attn, awm


# revision 57
# speedup vs baseline: 1.1213x; 1.0011x over previous
"""Trainium2 Bass kernel for nn_AttentionLayer_85383949844589.

Gated attention layer: B=16, C=K=128, D=256.
  g0 = BN0(q @ W0.T)          per-C-channel stats over (B, D)
  g1 = BN1(kc @ W1.T)         per-K-channel stats over (B, D)
  aw[b,c,k,d]   = sigmoid(g1)[b,k,d] * sigmoid(g0)[b,c,d]
  attn[b,c,k,d] = kc[b,k,d] * aw * cmask[b,c] * kmask[b,k]
  out[b,c,d]    = tanh(sum_k attn / klen[b])
  awm[b,c,k]    = mean_d aw

Sharding: the C (query-channel) axis is split across the 8 NeuronCores
(16 channels each).  BN0 stats are per-C-channel, so they are fully local
to a core; the g1/BN1 pipeline is replicated on every core (it is tiny).
No cross-core communication is needed at all.

Per core the dominant cost is writing its (B, C/8, K, D) = 32 MiB slice of
attn, i.e. the kernel is HBM-write-bound (~95 us at ~358 GB/s/core).

The big product is computed with D on the partition axis:
  big_t[d, c, k] = QgT[d, c] * A_t[d, k]
where QgT = sigmoid(g0)*cmask transposed and A_t = sigmoid(g1)T * (kc*kmask)T.
In that layout BOTH operands of the (C/8 x K) outer product are plain
free-dimension broadcast views (stride-0 free dims), so the DVE computes the
whole 4D block with two tensor_tensor ops per batch — no partition broadcast
is needed anywhere.  As a bonus the (d-partition, (c,k)-free) store has
8 KB-contiguous DRAM runs (vs 1 KB for the natural layout); the host
re-transposes the (B, 2, 128, C/8, K) device output once at the end.

Phase 1 (Y = x@W.T + per-channel sum/sumsq) runs on PE/scalar/vector with a
dep-free bf16 warm-up burst to lift the PE HAM throttle; phase 2 alternates
the two HWDGE queues so the 2 MB stores never head-of-line-block the
pipeline's small DMAs.  BN finalize runs split across scalar (BN1) and
vector (BN0) so the two serial chains overlap.
"""

import sys

sys.path.insert(0, "/opt/trn_rl_repo")

import numpy as np

B, C, K, D = 16, 128, 128, 256
NCORES = 8
CL = C // NCORES  # 16 query channels per core
EPS = 1e-5

_CACHE: dict = {}


def _build_nc():
    import concourse.tile as tile
    from concourse import bacc, mybir

    fp32 = mybir.dt.float32
    AF = mybir.ActivationFunctionType
    OP = mybir.AluOpType
    AX = mybir.AxisListType

    nc = bacc.Bacc(trn_type="TRN2", debug=False, num_devices=NCORES)

    # ---- DRAM I/O ----
    # qt[p, b, h, c]  = q[b, c_slice[c], h*128+p]
    qt_d = nc.dram_tensor("qt", [128, B, 2, CL], fp32, kind="ExternalInput")
    # kct[p, b, h, k] = kc[b, k, h*128+p]
    kct_d = nc.dram_tensor("kct", [128, B, 2, K], fp32, kind="ExternalInput")
    # kcm[p, b, h, k] = kc[b, k, h*128+p] * kmask[b, k]
    kcm_d = nc.dram_tensor("kcm", [128, B, 2, K], fp32, kind="ExternalInput")
    # wXt[p, h, o]    = WX[o, h*128+p]
    w0t_d = nc.dram_tensor("w0t", [128, 2, D], fp32, kind="ExternalInput")
    w1t_d = nc.dram_tensor("w1t", [128, 2, D], fp32, kind="ExternalInput")
    # all small constants packed into one tensor (single DMA):
    # cols [0]=g1, [1]=b1, [2]=g0(rows 0:CL), [3]=b0(rows 0:CL),
    # [4:4+B]=ilen(128,B), [20:20+B]=cmt(rows 0:CL), [36:164]=identity
    cpk_d = nc.dram_tensor("cpk", [128, 164], fp32, kind="ExternalInput")

    # transposed layouts (d on partitions); host reassembles
    ores_d = nc.dram_tensor("o_res", [B, 2, 128, CL], fp32, kind="ExternalOutput")
    attn_d = nc.dram_tensor("o_attn", [B, 2, 128, CL, K], fp32, kind="ExternalOutput")
    awm_d = nc.dram_tensor("o_awm", [B, CL, K], fp32, kind="ExternalOutput")

    BD = float(B * D)

    with tile.TileContext(nc) as tc:
        with (
            tc.tile_pool(name="const", bufs=1) as cp,
            tc.tile_pool(name="persist", bufs=1) as pp,
            tc.tile_pool(name="stats", bufs=1) as sp,
            tc.tile_pool(name="work", bufs=2) as wp,
            tc.tile_pool(name="bigout", bufs=4) as bp,
        ):
            # ---- load inputs; kct/kcm/w first so phase 1 starts ASAP ----
            kct = pp.tile([128, B * 2 * K], fp32)
            kcm = pp.tile([128, B * 2 * K], fp32)
            CH = B // 4

            def load_kct(i):
                eng = nc.sync if i % 2 == 0 else nc.scalar
                sl_d = slice(i * CH, (i + 1) * CH)
                sl_s = slice(i * CH * 2 * K, (i + 1) * CH * 2 * K)
                eng.dma_start(
                    kct[:, sl_s],
                    kct_d.ap()[:, sl_d].rearrange("p b h k -> p (b h k)"),
                )

            def load_kcm(i):
                eng = nc.sync if i % 2 == 0 else nc.scalar
                sl_d = slice(i * CH, (i + 1) * CH)
                sl_s = slice(i * CH * 2 * K, (i + 1) * CH * 2 * K)
                eng.dma_start(
                    kcm[:, sl_s],
                    kcm_d.ap()[:, sl_d].rearrange("p b h k -> p (b h k)"),
                )

            load_kct(0)
            load_kct(1)
            w1t = cp.tile([128, 2 * D], fp32)
            nc.sync.dma_start(w1t[:], w1t_d.ap().rearrange("p h o -> p (h o)"))
            w0t = cp.tile([128, 2 * D], fp32)
            nc.scalar.dma_start(w0t[:], w0t_d.ap().rearrange("p h o -> p (h o)"))
            load_kct(2)
            load_kct(3)
            qt = pp.tile([128, B * 2 * CL], fp32)
            nc.scalar.dma_start(qt[:], qt_d.ap().rearrange("p b h c -> p (b h c)"))
            cpk = cp.tile([128, 164], fp32)
            nc.sync.dma_start(cpk[:], cpk_d.ap()[:])
            for i in range(4):
                load_kcm(i)
            g1c = cpk[:, 0:1]
            b1c = cpk[:, 1:2]
            g0c = cpk[0:CL, 2:3]
            b0c = cpk[0:CL, 3:4]
            ilen = cpk[:, 4 : 4 + B]
            cmt = cpk[0:CL, 20 : 20 + B]
            iden = cpk[:, 36:164]

            y1sb = pp.tile([K, B * D], fp32)
            y0sb = pp.tile([CL, B * D], fp32)

            s1cols = sp.tile([K, B], fp32)
            q1cols = sp.tile([K, B], fp32)
            s0cols = sp.tile([CL, B], fp32)
            q0cols = sp.tile([CL, B], fp32)
            sq1s = sp.tile([K, 2 * D], fp32)
            sq0s = sp.tile([CL, 2 * D], fp32)
            epst = sp.tile([128, 1], fp32)
            nc.vector.memset(epst[:], EPS)
            ores_all = pp.tile([128, B * 2 * CL], fp32)
            awm_all = pp.tile([CL, B * K], fp32)

            # ---- PE warm-up burst ----
            # HAM starts the PE throttled (1.2 GHz) and only un-throttles
            # after a sustained-busy window.  Run dep-free bf16 matmuls while
            # the input DMAs land so phase 1 runs at 2.4 GHz.
            bf16 = mybir.dt.bfloat16
            wu_a = sp.tile([128, 128], bf16)
            nc.vector.memset(wu_a[:], 1.0)
            wu_b = sp.tile([128, 512], bf16)
            nc.vector.memset(wu_b[:], 1.0)
            # ---- phase 1: Y0/Y1 matmuls + per-channel sum / sumsq ----
            with tc.tile_pool(name="ps1", bufs=3, space="PSUM") as ps1:
                # warm-up shares the pool so phase 1 isn't serialized behind
                # a pool close; sized to end right as the first inputs land
                wu_ps = ps1.tile([128, 512], fp32, tag="wu", bufs=1)
                NWU = 16
                for i in range(NWU):
                    nc.tensor.matmul(
                        wu_ps[:], wu_a[:], wu_b[:],
                        start=(i == 0), stop=(i == NWU - 1),
                    )
                wu_out = sp.tile([1, 1], fp32)
                nc.scalar.copy(wu_out[:], wu_ps[0:1, 0:1])

                for b in range(B):
                    y1ps = ps1.tile([K, D], fp32, tag="y1ps")
                    for h in range(2):
                        nc.tensor.matmul(
                            y1ps[:],
                            kct[:, b * 256 + h * 128 : b * 256 + h * 128 + 128],
                            w1t[:, h * D : (h + 1) * D],
                            start=(h == 0),
                            stop=(h == 1),
                        )
                    nc.scalar.copy(y1sb[:, b * D : (b + 1) * D], y1ps[:])

                    y0ps = ps1.tile([CL, D], fp32, tag="y0ps")
                    for h in range(2):
                        nc.tensor.matmul(
                            y0ps[:],
                            qt[:, b * 2 * CL + h * CL : b * 2 * CL + (h + 1) * CL],
                            w0t[:, h * D : (h + 1) * D],
                            start=(h == 0),
                            stop=(h == 1),
                        )
                    nc.scalar.copy(y0sb[:, b * D : (b + 1) * D], y0ps[:])

                    # per-2-batch channel stats on the vector engine, from
                    # the SBUF copies (vector must not touch live PSUM here)
                    if b % 2 == 1:
                        i4 = b // 2
                        c0, c1 = (b - 1) * D, (b + 1) * D
                        nc.vector.tensor_reduce(
                            s1cols[:, i4 : i4 + 1], y1sb[:, c0:c1], AX.X, OP.add
                        )
                        nc.vector.tensor_mul(sq1s[:], y1sb[:, c0:c1], y1sb[:, c0:c1])
                        nc.vector.tensor_reduce(
                            q1cols[:, i4 : i4 + 1], sq1s[:], AX.X, OP.add
                        )
                        nc.vector.tensor_reduce(
                            s0cols[:, i4 : i4 + 1], y0sb[:, c0:c1], AX.X, OP.add
                        )
                        nc.vector.tensor_mul(sq0s[:], y0sb[:, c0:c1], y0sb[:, c0:c1])
                        nc.vector.tensor_reduce(
                            q0cols[:, i4 : i4 + 1], sq0s[:], AX.X, OP.add
                        )

            # ---- phase boundary: finalize BN scale/shift ----
            # s = gamma / sqrt(var+eps);  t = beta - mean * s
            # Mostly on the scalar engine: DVE ops pay a pipeline DRAIN each,
            # which dominates this serial chain of tiny (P,1) ops.
            def bn_finalize(P, scols, qcols, gc, bc):
                ssum = sp.tile([P, 1], fp32, name=f"ssum{P}")
                nc.vector.tensor_reduce(ssum[:], scols[:, 0:8], AX.X, OP.add)
                qsum = sp.tile([P, 1], fp32, name=f"qsum{P}")
                nc.vector.tensor_reduce(qsum[:], qcols[:, 0:8], AX.X, OP.add)
                mean = sp.tile([P, 1], fp32, name=f"mean{P}")
                nc.scalar.mul(mean[:], ssum[:], 1.0 / BD)
                # ex2e = E[x^2] + eps
                ex2e = sp.tile([P, 1], fp32, name=f"ex2e{P}")
                nc.scalar.activation(
                    ex2e[:], qsum[:], AF.Identity, bias=epst[:P], scale=1.0 / BD
                )
                msq = sp.tile([P, 1], fp32, name=f"msq{P}")
                nc.scalar.square(msq[:], mean[:])
                # varp = ex2e - mean^2
                varp = sp.tile([P, 1], fp32, name=f"varp{P}")
                nc.scalar.activation(
                    varp[:], msq[:], AF.Identity, bias=ex2e[:], scale=-1.0
                )
                std = sp.tile([P, 1], fp32, name=f"std{P}")
                nc.scalar.sqrt(std[:], varp[:])
                # one Newton step to clean up the scalar-engine sqrt:
                # std' = 0.5*(std + varp/std)
                rstd = sp.tile([P, 1], fp32, name=f"rstd{P}")
                nc.vector.reciprocal(rstd[:], std[:])
                qh = sp.tile([P, 1], fp32, name=f"qh{P}")
                nc.scalar.mul(qh[:], varp[:], rstd[:])  # varp/std
                stdh = sp.tile([P, 1], fp32, name=f"stdh{P}")
                nc.scalar.mul(stdh[:], std[:], 0.5)
                std2 = sp.tile([P, 1], fp32, name=f"std2{P}")
                nc.scalar.activation(
                    std2[:], qh[:], AF.Identity, bias=stdh[:], scale=0.5
                )
                inv = sp.tile([P, 1], fp32, name=f"inv{P}")
                nc.vector.reciprocal(inv[:], std2[:])
                s_ = sp.tile([P, 1], fp32, name=f"s_{P}")
                nc.scalar.mul(s_[:], inv[:], gc[:])
                ms = sp.tile([P, 1], fp32, name=f"ms{P}")
                nc.scalar.mul(ms[:], mean[:], s_[:])
                t_ = sp.tile([P, 1], fp32, name=f"t_{P}")
                nc.scalar.activation(
                    t_[:], ms[:], AF.Identity, bias=bc[:], scale=-1.0
                )
                return s_, t_

                # bridge burst: keeps the PE HAM lit across the BN boundary
                # (the real transposes are data-gated for ~6 us anyway)
                wu2_ps = ps1.tile([128, 512], fp32, tag="wu", bufs=1)
                for i in range(8):
                    nc.tensor.matmul(
                        wu2_ps[:], wu_a[:], wu_b[:],
                        start=(i == 0), stop=(i == 7),
                    )
                wu2_out = sp.tile([1, 1], fp32)
                nc.scalar.copy(wu2_out[:], wu2_ps[0:1, 0:1])

            def bn_finalize_v(P, scols, qcols, gc, bc):
                # vector-engine variant so BN0 finalizes concurrently with
                # BN1 on the scalar engine
                ssum = sp.tile([P, 1], fp32, name=f"vssum{P}")
                nc.vector.tensor_reduce(ssum[:], scols[:, 0:8], AX.X, OP.add)
                qsum = sp.tile([P, 1], fp32, name=f"vqsum{P}")
                nc.vector.tensor_reduce(qsum[:], qcols[:, 0:8], AX.X, OP.add)
                mean = sp.tile([P, 1], fp32, name=f"vmean{P}")
                nc.vector.tensor_scalar_mul(mean[:], ssum[:], 1.0 / BD)
                ex2e = sp.tile([P, 1], fp32, name=f"vex2e{P}")
                nc.vector.tensor_scalar(
                    ex2e[:], qsum[:], 1.0 / BD, EPS, OP.mult, OP.add
                )
                msq = sp.tile([P, 1], fp32, name=f"vmsq{P}")
                nc.vector.tensor_mul(msq[:], mean[:], mean[:])
                varp = sp.tile([P, 1], fp32, name=f"vvarp{P}")
                nc.vector.tensor_sub(varp[:], ex2e[:], msq[:])
                std = sp.tile([P, 1], fp32, name=f"vstd{P}")
                nc.scalar.sqrt(std[:], varp[:])
                rstd = sp.tile([P, 1], fp32, name=f"vrstd{P}")
                nc.vector.reciprocal(rstd[:], std[:])
                q_ = sp.tile([P, 1], fp32, name=f"vq_{P}")
                nc.vector.tensor_mul(q_[:], varp[:], rstd[:])
                nc.vector.tensor_add(std[:], std[:], q_[:])
                nc.vector.tensor_scalar_mul(std[:], std[:], 0.5)
                inv = sp.tile([P, 1], fp32, name=f"vinv{P}")
                nc.vector.reciprocal(inv[:], std[:])
                s_ = sp.tile([P, 1], fp32, name=f"vs_{P}")
                nc.vector.tensor_mul(s_[:], inv[:], gc[:])
                ms = sp.tile([P, 1], fp32, name=f"vms{P}")
                nc.vector.tensor_mul(ms[:], mean[:], s_[:])
                t_ = sp.tile([P, 1], fp32, name=f"vt_{P}")
                nc.vector.tensor_sub(t_[:], bc[:], ms[:])
                return s_, t_

            s1, t1 = bn_finalize(K, s1cols, q1cols, g1c, b1c)
            s0, t0 = bn_finalize_v(CL, s0cols, q0cols, g0c, b0c)

            # Bake cmask into a per-(c,b) scale/bias so Qg = sigmoid-masked
            # comes straight off the scalar engine:
            #   masked: sigmoid(s0*y + t0);  unmasked: sigmoid(0*y - 1e30) = 0
            s0b = sp.tile([CL, B], fp32)
            nc.vector.tensor_scalar(s0b[:], cmt[:], s0[:], None, OP.mult)
            t0b = sp.tile([CL, B], fp32)
            # t0b = t0*cm + (cm-1)*1e30
            nc.vector.tensor_scalar(t0b[:], cmt[:], 1.0, 1e30, OP.subtract, OP.mult)
            tb2 = sp.tile([CL, B], fp32)
            nc.vector.tensor_scalar(tb2[:], cmt[:], t0[:], None, OP.mult)
            nc.vector.tensor_add(t0b[:], t0b[:], tb2[:])

            # ---- phase 2 ----
            # PSUM transpose-staging layout (single bank):
            #   [  0:128) sig1T h0   [128:256) sig1T h1
            #   [256:272) sig0T h0   [272:288) sig0T h1
            #   [288:304) QgT  h0    [304:320) QgT  h1
            S1T, S0T, QGT = 0, 256, 288
            with (
                tc.tile_pool(name="pst", bufs=2, space="PSUM") as pst,  # transposes
                tc.tile_pool(name="psr", bufs=2, space="PSUM") as psr,  # awm
            ):
                for b in range(B):
                    # alternate HWDGE queues so the big output DMA never
                    # head-of-line-blocks the small pipeline DMAs
                    dq = nc.sync if (b % 2 == 0) else nc.scalar
                    oq = nc.scalar if (b % 2 == 0) else nc.sync

                    yb = y1sb[:, b * D : (b + 1) * D]
                    sig1 = wp.tile([K, D], fp32, tag="sig1")
                    nc.scalar.activation(
                        sig1[:], yb, AF.Sigmoid, bias=t1[:], scale=s1[:]
                    )

                    sig0 = wp.tile([CL, D], fp32, tag="sig0")
                    nc.scalar.activation(
                        sig0[:],
                        y0sb[:, b * D : (b + 1) * D],
                        AF.Sigmoid,
                        bias=t0[:],
                        scale=s0[:],
                    )
                    qg = wp.tile([CL, D], fp32, tag="qg")
                    nc.scalar.activation(
                        qg[:],
                        y0sb[:, b * D : (b + 1) * D],
                        AF.Sigmoid,
                        bias=t0b[:, b : b + 1],
                        scale=s0b[:, b : b + 1],
                    )

                    # transpose into d-on-partitions layout
                    tps = pst.tile([128, 320], fp32, tag="tps")
                    for h in range(2):
                        nc.tensor.transpose(
                            tps[:, S1T + h * K : S1T + (h + 1) * K],
                            sig1[:, h * 128 : (h + 1) * 128],
                            iden[:, 0:128],
                        )
                        nc.tensor.transpose(
                            tps[:, S0T + h * CL : S0T + (h + 1) * CL],
                            sig0[:, h * 128 : (h + 1) * 128],
                            iden[0:CL, 0:CL],
                        )
                        nc.tensor.transpose(
                            tps[:, QGT + h * CL : QGT + (h + 1) * CL],
                            qg[:, h * 128 : (h + 1) * 128],
                            iden[0:CL, 0:CL],
                        )
                    st = wp.tile([128, 320], fp32, tag="st")
                    nc.scalar.copy(st[:], tps[:])


                    # A_t[d, k] = sig1T[d, k] * (kc*kmask)T[d, k];
                    # accum_out gives sum_k A_t = the attention-vector sum
                    at2 = wp.tile([128, 2 * K], fp32, tag="at2")
                    sA = wp.tile([128, 2], fp32, tag="sA")
                    for h in range(2):
                        nc.vector.scalar_tensor_tensor(
                            at2[:, h * K : (h + 1) * K],
                            st[:, S1T + h * K : S1T + (h + 1) * K],
                            1.0,
                            kcm[:, b * 2 * K + h * K : b * 2 * K + (h + 1) * K],
                            op0=OP.bypass,
                            op1=OP.mult,
                            accum_out=sA[:, h : h + 1],
                        )

                    # awm[c,k] = (1/D) * sum_d sig0T[d,c] * sig1T[d,k]
                    psr_t = psr.tile([CL, K], fp32, tag="psr")
                    for h in range(2):
                        nc.tensor.matmul(
                            psr_t[:],
                            st[:, S0T + h * CL : S0T + (h + 1) * CL],
                            st[:, S1T + h * K : S1T + (h + 1) * K],
                            start=(h == 0),
                            stop=(h == 1),
                        )
                    nc.scalar.mul(
                        awm_all[:, b * K : (b + 1) * K], psr_t[:], 1.0 / D
                    )

                    # attention_vector (transposed): sum_k A_t along free,
                    # then av_t[d,c] = QgT[d,c] * sumA[d], tanh(av/klen)
                    av_t = wp.tile([128, 2 * CL], fp32, tag="av_t")
                    for h in range(2):
                        nc.scalar.mul(
                            av_t[:, h * CL : (h + 1) * CL],
                            st[:, QGT + h * CL : QGT + (h + 1) * CL],
                            sA[:, h : h + 1],
                        )
                    nc.scalar.activation(
                        ores_all[:, b * 2 * CL : (b + 1) * 2 * CL],
                        av_t[:],
                        AF.Tanh,
                        bias=0.0,
                        scale=ilen[:, b : b + 1],
                    )

                    # big product, d on partitions:
                    #   big_t[d, c, k] = QgT[d, c] * A_t[d, k]
                    big = bp.tile([128, 2 * CL * K], fp32, tag="big")
                    nc.vector.tensor_tensor(
                        big[:].rearrange("p (h c k) -> p h c k", c=CL, k=K),
                        st[:, QGT : QGT + 2 * CL]
                        .rearrange("p (h c) -> p h c", c=CL)
                        .unsqueeze(3)
                        .to_broadcast([128, 2, CL, K]),
                        at2[:]
                        .rearrange("p (h k) -> p h k", k=K)
                        .unsqueeze(2)
                        .to_broadcast([128, 2, CL, K]),
                        OP.mult,
                    )
                    if b < B - 1:
                        dq.dma_start(
                            attn_d.ap()[b].rearrange("h p c k -> p h c k"),
                            big[:].rearrange("p (h c k) -> p h c k", c=CL, k=K),
                        )
                    else:
                        # tail: split the final store across both queues so
                        # the drain barrier isn't gated on one 2 MB stream
                        nc.sync.dma_start(
                            attn_d.ap()[b, 0],
                            big[:, 0 : CL * K].rearrange("p (c k) -> p c k", k=K),
                        )
                        nc.scalar.dma_start(
                            attn_d.ap()[b, 1],
                            big[:, CL * K : 2 * CL * K].rearrange(
                                "p (c k) -> p c k", k=K
                            ),
                        )

                if True:
                    nc.sync.dma_start(
                        ores_d.ap().rearrange("b h p c -> p b h c"),
                        ores_all[:].rearrange("p (b h c) -> p b h c", h=2, c=CL),
                    )
                    nc.scalar.dma_start(
                        awm_d.ap().rearrange("b c k -> c b k"),
                        awm_all[:].rearrange("c (b k) -> c b k", k=K),
                    )

    nc.compile()
    return nc


def _get_nc():
    if "nc" not in _CACHE:
        _CACHE["nc"] = _build_nc()
    return _CACHE["nc"]


def _make_in_maps(inputs):
    q = np.ascontiguousarray(inputs["query_candidates_repr"], dtype=np.float32)
    kc = np.ascontiguousarray(inputs["key_candidates"], dtype=np.float32)
    W0 = np.asarray(inputs["W0"], dtype=np.float32)
    W1 = np.asarray(inputs["W1"], dtype=np.float32)
    g0 = np.asarray(inputs["bn0_gamma"], dtype=np.float32)
    b0 = np.asarray(inputs["bn0_beta"], dtype=np.float32)
    g1 = np.asarray(inputs["bn1_gamma"], dtype=np.float32)
    b1 = np.asarray(inputs["bn1_beta"], dtype=np.float32)
    cm = np.asarray(inputs["query_candidate_mask"]).astype(np.float32)
    km = np.asarray(inputs["key_candidate_mask"]).astype(np.float32)
    kl = np.asarray(inputs["key_candidate_len"]).astype(np.float32)

    kct = np.ascontiguousarray(
        kc.reshape(B, K, 2, 128).transpose(3, 0, 2, 1)
    )  # (128, B, 2, K)
    kcm = np.ascontiguousarray(
        (kc * km[:, :, None]).reshape(B, K, 2, 128).transpose(3, 0, 2, 1)
    )  # (128, B, 2, K), kmask folded in
    w0t = np.ascontiguousarray(W0.reshape(D, 2, 128).transpose(2, 1, 0))
    w1t = np.ascontiguousarray(W1.reshape(D, 2, 128).transpose(2, 1, 0))

    shared = dict(kct=kct, kcm=kcm, w0t=w0t, w1t=w1t)
    in_maps = []
    for r in range(NCORES):
        sl = slice(r * CL, (r + 1) * CL)
        qt = np.ascontiguousarray(
            q[:, sl, :].reshape(B, CL, 2, 128).transpose(3, 0, 2, 1)
        )
        cpk = np.zeros((128, 164), np.float32)
        cpk[:, 0] = g1
        cpk[:, 1] = b1
        cpk[:CL, 2] = g0[sl]
        cpk[:CL, 3] = b0[sl]
        cpk[:, 4 : 4 + B] = np.tile(1.0 / kl, (128, 1))
        cpk[:CL, 20 : 20 + B] = cm[:, sl].T
        cpk[:, 36:164] = np.eye(128, dtype=np.float32)
        m = dict(shared, qt=qt, cpk=cpk)
        in_maps.append(m)
    return in_maps


def run(inputs, trace=False):
    from concourse import bass_utils

    nc = _get_nc()
    in_maps = _make_in_maps(inputs)
    res = bass_utils.run_bass_kernel_spmd(
        nc, in_maps, core_ids=list(range(NCORES)), trace=trace
    )
    # device outputs are d-on-partitions (B, 2, 128, CL[, K]); restore layout
    ores_t = np.stack([res.results[r]["o_res"] for r in range(NCORES)], axis=3)
    # (B, 2, 128, NCORES, CL) -> (B, C, D)
    out_res = np.ascontiguousarray(
        ores_t.transpose(0, 3, 4, 1, 2).reshape(B, C, D)
    )
    attn_t = np.stack([res.results[r]["o_attn"] for r in range(NCORES)], axis=3)
    # (B, 2, 128, NCORES, CL, K) -> (B, C, K, D)
    attn = np.ascontiguousarray(
        attn_t.transpose(0, 3, 4, 5, 1, 2).reshape(B, C, K, D)
    )
    awm = np.concatenate([res.results[r]["o_awm"] for r in range(NCORES)], axis=1)
    return (out_res, attn, awm), res


def kernel(**inputs):
    (out_res, attn, awm), _ = run(inputs, trace=False)
    return out_res, attn, awm
